# revision 1
# baseline (speedup 1.0000x reference)
"""Multi-head causal attention (B=2, S=2048, D=1024, H=16) on 8 TRN2 NeuronCores.
135.3us TimelineSim (baseline 184.4us).

Sharding: core c handles batch b = c // 4 and local head group g = c % 4
(global heads 4g..4g+3).  Each core computes its heads' QKV projections,
causal attention, and a partial output projection; host sums the 4 partials
per batch and adds b_out.

v2 design (vs baseline):
  - QK projection: fp8e4 DoubleRow matmuls, 3-pass hi/lo error compensation
    (x8@W8 + x8lo@W8 + x8@W8lo) -> q,k accurate to ~0.4%; f32r scores.
  - V projection: same 3-pass fp8 DR -> vn in bf16 (k-major, +ones col).
  - Scores: f32r, two heads packed per 128-partition psum tile, causal
    trimming at 256 granularity (f32r needs moving dim >= 256).
  - exp on ACT -> bf16 E tiles; triangle masks on DVE (bf16 2x mode).
  - AV transposed: out[q(128), 65] = E_block[k,q].T @ Vn[k, 65]; moving dim
    is only 65 cols -> ~2x fewer PE cycles than value-major AV.  Ones column
    of Vn gives the softmax denominator per q ON THE PARTITION, so
    normalization is a per-partition reciprocal + tensor_scalar multiply
    (no cross-partition broadcast needed at all).
  - values transposed back to [d, q] with PE transpose matmuls (bf16),
    bf16 output projection.
"""

from contextlib import ExitStack

import numpy as np
import ml_dtypes

import concourse.bass as bass
import concourse.mybir as mybir
import concourse.tile as tile
from concourse import bass_utils

F32 = mybir.dt.float32
F32R = mybir.dt.float32r
BF16 = mybir.dt.bfloat16
FP8 = mybir.dt.float8e4
EXP = mybir.ActivationFunctionType.Exp
COPY = mybir.ActivationFunctionType.Copy
DR = mybir.MatmulPerfMode.DoubleRow

E4 = ml_dtypes.float8_e4m3
BF = ml_dtypes.bfloat16

B, S, D, H = 2, 2048, 1024, 16
HD = D // H          # 64
HL = 4               # heads per core
N_CORES = 8
SC = S // 512        # 4 q-chunks of 512
KT = S // 128        # 16 k-tiles of 128

_CACHE = {}


def _round_f32r(x: np.ndarray) -> np.ndarray:
    """Round f32 to fp32r (11-bit mantissa, RNE) on host."""
    u = np.ascontiguousarray(x, dtype=np.float32).view(np.uint32)
    frac = u & np.uint32(0x00000FFF)
    base = u & np.uint32(0xFFFFF000)
    bit = np.uint32(0x00000800)
    lsb = np.uint32(0x00001000)
    roundup = (frac > bit) | ((frac == bit) & ((u & lsb) != 0))
    return np.where(roundup, base + lsb, base).view(np.float32)


_NO_HOIST = {
    "AllEngineBarrier",
    "EventSemaphore",
    "UnconditionalBranch",
    "CompareAndBranch",
    "BranchHint",
    "IndirectBranch",
    "Halt",
    "Call",
    "OverlayCall",
    "NoOp",
}


def _fix_sync_waits(nc):
    """walrus codegen holds only one sync-wait per engine instruction; hoist
    excess waits onto same-engine NoOps inserted right before."""
    for fn in nc.m.functions:
        for blk in fn.blocks:
            insts = blk.instructions
            out = []
            changed = False
            for inst in insts:
                si = inst.sync_info
                if si is not None and inst.opcode not in _NO_HOIST:
                    waits = list(si.on_wait)
                    if len(waits) > 1:
                        for j, w in enumerate(waits[:-1]):
                            nop = mybir.InstNoOp(name=f"{inst.name}-wfix{j}")
                            nop.engine = inst.engine
                            nop.sync_info = mybir.SyncInfo(on_wait=[w], on_update=[])
                            out.append(nop)
                        inst.sync_info = mybir.SyncInfo(
                            on_wait=[waits[-1]], on_update=list(si.on_update)
                        )
                        changed = True
                out.append(inst)
            if changed:
                blk.instructions = out


def _build(fix_waits=True, dbg=False):
    nc = bass.Bass("TRN2", target_bir_lowering=False, debug=False,
                   num_devices=N_CORES)
    if dbg:
        d_qT = nc.dram_tensor("d_qT", [128, 2, S], F32R, kind="ExternalOutput").ap()
        d_kT = nc.dram_tensor("d_kT", [128, 2, S], F32R, kind="ExternalOutput").ap()
        d_vn = nc.dram_tensor("d_vn", [128, KT, 4, 65], BF16,
                              kind="ExternalOutput").ap()
        d_e = nc.dram_tensor("d_e", [128, 2, 512], BF16, kind="ExternalOutput").ap()
        d_vst = nc.dram_tensor("d_vst", [128, 4, 4, 64], BF16,
                               kind="ExternalOutput").ap()
        d_vnT = nc.dram_tensor("d_vnT", [128, 2, S], BF16,
                               kind="ExternalOutput").ap()

    xq8 = nc.dram_tensor("xq8", [128, 4, 2, S], FP8, kind="ExternalInput").ap()
    xq8l = nc.dram_tensor("xq8l", [128, 4, 2, S], FP8, kind="ExternalInput").ap()
    xs8 = nc.dram_tensor("xs8", [128, 4, 2, S], FP8, kind="ExternalInput").ap()
    zro = nc.dram_tensor("zro", [128, 384], F32R, kind="ExternalInput").ap()
    wq8 = nc.dram_tensor("wq8", [128, 4, 2, 4, 128], FP8, kind="ExternalInput").ap()
    wq8l = nc.dram_tensor("wq8l", [128, 4, 2, 4, 128], FP8, kind="ExternalInput").ap()
    wq8s = nc.dram_tensor("wq8s", [128, 4, 2, 4, 128], FP8, kind="ExternalInput").ap()
    wv8 = nc.dram_tensor("wv8", [128, 4, 2, 256], FP8, kind="ExternalInput").ap()
    wv8l = nc.dram_tensor("wv8l", [128, 4, 2, 256], FP8, kind="ExternalInput").ap()
    wv8s = nc.dram_tensor("wv8s", [128, 4, 2, 256], FP8, kind="ExternalInput").ap()
    woutb = nc.dram_tensor("woutb", [128, 2, D], BF16, kind="ExternalInput").ap()
    bq = nc.dram_tensor("bq", [128, 4], F32, kind="ExternalInput").ap()
    bv = nc.dram_tensor("bv", [128, 4, 64], F32, kind="ExternalInput").ap()
    vone = nc.dram_tensor("vone", [128, KT, 4, 1], BF16, kind="ExternalInput").ap()
    cmask = nc.dram_tensor("cmask", [128, 128], BF16, kind="ExternalInput").ap()
    identb = nc.dram_tensor("identb", [128, 128], BF16, kind="ExternalInput").ap()
    outT = nc.dram_tensor("outT", [128, 8, S], BF16, kind="ExternalOutput").ap()

    with tile.TileContext(nc) as tc, ExitStack() as ctx:
        persist = ctx.enter_context(tc.tile_pool(name="persist", bufs=1))
        xpool = ctx.enter_context(tc.tile_pool(name="xp", bufs=3))
        epool = ctx.enter_context(tc.tile_pool(name="ep", bufs=8))
        spool = ctx.enter_context(tc.tile_pool(name="stp", bufs=3))
        opool = ctx.enter_context(tc.tile_pool(name="op", bufs=6))
        # psum (8 banks): sp 2x2-bank, po 1x2-bank, small (pq/pv/pu/tr) 2x1
        ps = ctx.enter_context(tc.tile_pool(name="ps", bufs=2, space="PSUM"))

        wq_sb = persist.tile([128, 4, 2, 4, 128], FP8, tag="wq")
        wql_sb = persist.tile([128, 4, 2, 4, 128], FP8, tag="wql")
        wqs_sb = persist.tile([128, 4, 2, 4, 128], FP8, tag="wqs")
        wv_sb = persist.tile([128, 4, 2, 256], FP8, tag="wv")
        wvl_sb = persist.tile([128, 4, 2, 256], FP8, tag="wvl")
        wvs_sb = persist.tile([128, 4, 2, 256], FP8, tag="wvs")
        zro_sb = persist.tile([128, 384], F32R, tag="zro")
        wo_sb = persist.tile([128, 2, D], BF16, tag="wo")
        bq_sb = persist.tile([128, 4], F32, tag="bq")
        bv_sb = persist.tile([128, 4, 64], F32, tag="bv")
        cm_sb = persist.tile([128, 128], BF16, tag="cm")
        id_sb = persist.tile([128, 128], BF16, tag="id")
        qT = persist.tile([128, 2, S], F32R, tag="qT")
        kT = persist.tile([128, 2, S], F32R, tag="kT")
        vn = persist.tile([128, KT, 4, 65], BF16, tag="vn")
        vnT = persist.tile([128, 2, S], BF16, tag="vnT")

        # first x chunk first so the first matmuls start early
        xc0 = xpool.tile([128, 4, 2, 512], FP8, tag="xc", name="xc0")
        xl0 = xpool.tile([128, 4, 2, 512], FP8, tag="xl", name="xl0")
        xs0 = xpool.tile([128, 4, 2, 512], FP8, tag="xs", name="xs0")
        nc.sync.dma_start(xc0[:], xq8[:, :, :, 0:512])
        nc.scalar.dma_start(bq_sb[:], bq)
        nc.scalar.dma_start(wq_sb[:], wq8)
        nc.sync.dma_start(xs0[:], xs8[:, :, :, 0:512])
        nc.scalar.dma_start(wql_sb[:], wq8l)
        nc.sync.dma_start(xl0[:], xq8l[:, :, :, 0:512])
        nc.scalar.dma_start(wqs_sb[:], wq8s)
        nc.scalar.dma_start(bv_sb[:], bv)
        nc.scalar.dma_start(wv_sb[:], wv8)
        nc.scalar.dma_start(wvl_sb[:], wv8l)
        nc.scalar.dma_start(wvs_sb[:], wv8s)
        nc.scalar.dma_start(cm_sb[:], cmask)
        nc.scalar.dma_start(zro_sb[:], zro)
        nc.scalar.dma_start(id_sb[:], identb)
        nc.scalar.dma_start(wo_sb[:], woutb)
        # ones column of vn (softmax denominators) via memset, not DMA
        # (a strided single-element-column DMA costs ~3.6us of DMA engines)
        nc.vector.memset(vn[:, :, :, 64:65], 1.0)
        xtiles = {0: (xc0, xl0, xs0)}

        def qkv_dma(qc):
            qs = slice(qc * 512, (qc + 1) * 512)
            xc = xpool.tile([128, 4, 2, 512], FP8, tag="xc", name=f"xc{qc}")
            xl = xpool.tile([128, 4, 2, 512], FP8, tag="xl", name=f"xl{qc}")
            xs = xpool.tile([128, 4, 2, 512], FP8, tag="xs", name=f"xs{qc}")
            nc.sync.dma_start(xc[:], xq8[:, :, :, qs])
            nc.sync.dma_start(xl[:], xq8l[:, :, :, qs])
            nc.sync.dma_start(xs[:], xs8[:, :, :, qs])
            xtiles[qc] = (xc, xl, xs)

        def qk_tile(qc, mt):
            qs = slice(qc * 512, (qc + 1) * 512)
            xc, xl, xs = xtiles[qc]
            pq = ps.tile([128, 512], F32, tag="q1", name=f"pq{qc}{mt}")
            passes = [(wq_sb, xc), (wql_sb, xs), (wqs_sb, xl)]
            i = 0
            for wsb, xsb in passes:
                for kp in range(4):
                    nc.tensor.matmul(
                        pq[:], wsb[:, kp, :, mt, :], xsb[:, kp, :, :],
                        start=(i == 0), stop=(i == 11), perf_mode=DR)
                    i += 1
            dst = (qT if mt < 2 else kT)[:, mt % 2, qs]
            nc.vector.tensor_scalar_add(dst, pq[:], bq_sb[:, mt:mt + 1])

        def v_tile(qc, j):
            st = 4 * qc + j
            xc, xl, xs = xtiles[qc]
            pv = ps.tile([128, 512], F32, tag="q1", name=f"pv{qc}{j}")
            passes = [(wv_sb, xc), (wvl_sb, xs), (wvs_sb, xl)]
            i = 0
            for wsb, xsb in passes:
                for kp in range(4):
                    nc.tensor.matmul(
                        pv[0:128, 0:256], xsb[:, kp, :, j * 128:(j + 1) * 128],
                        wsb[:, kp, :, :],
                        start=(i == 0), stop=(i == 11), perf_mode=DR)
                    i += 1
            nc.vector.tensor_add(
                vn[:, st, :, 0:64],
                pv[0:128, 0:256].rearrange("p (h d) -> p h d", h=4),
                bv_sb[:])

        def qkv_early(qc):
            ps_ = [lambda qc=qc: qkv_dma(qc)] if qc > 0 else []
            for mt in (0, 2):
                ps_.append(lambda qc=qc, mt=mt: qk_tile(qc, mt))
            for j in (0, 1):
                ps_.append(lambda qc=qc, j=j: v_tile(qc, j))
            return ps_

        def qkv_late(qc):
            ps_ = [lambda qc=qc, j=j: v_tile(qc, j) for j in (2, 3)]
            for mt in (1, 3):
                ps_.append(lambda qc=qc, mt=mt: qk_tile(qc, mt))
            return ps_

        vst_tiles = {}

        def tr_piece(qc, qt):
            vst = vst_tiles[qc]
            for dh in range(2):
                ptr = ps.tile([128, 128], BF16, tag="q1", name=f"tr{qc}{qt}{dh}")
                nc.tensor.matmul(ptr[:], vst[:, qt, 2 * dh:2 * dh + 2, :],
                                 id_sb[:], is_transpose=True)
                nc.vector.tensor_copy(
                    vnT[:, dh, qc * 512 + qt * 128:qc * 512 + (qt + 1) * 128],
                    ptr[:])

        def op_piece(qc, m):
            qs = slice(qc * 512, (qc + 1) * 512)
            pu = ps.tile([128, 512], F32, tag="q1", name=f"pu{qc}{m}")
            for t in range(2):
                nc.tensor.matmul(pu[:], wo_sb[:, t, m * 128:(m + 1) * 128],
                                 vnT[:, t, qs], start=(t == 0), stop=(t == 1))
            ou = opool.tile([128, 512], BF16, tag="ou", name=f"ou{qc}{m}")
            if qc == SC - 1 and m % 2 == 1:
                nc.scalar.copy(ou[:], pu[:])   # tail: ACT is idle
            else:
                nc.vector.tensor_copy(ou[:], pu[:])
            nc.sync.dma_start(outT[:, m, qs], ou[:])

        def post_pieces(qc):
            ps_ = [lambda qc=qc, qt=qt: tr_piece(qc, qt) for qt in range(4)]
            ps_ += [lambda qc=qc, m=m: op_piece(qc, m) for m in range(8)]
            return ps_

        def attn_qc(qc, queue):
            """Emit attention for chunk qc, interleaving `queue` pieces (PE
            work for the next chunk's projections and the previous chunk's
            transposes/output projection) between ki steps so every engine
            stays fed while the exp (ACT) chain runs."""
            vst = spool.tile([128, 4, 4, 64], BF16, tag="vst", name=f"vs{qc}")
            vst_tiles[qc] = vst
            n_ki = 4 * qc + 4
            n_steps = 2 * n_ki + 2
            qi = 0
            emitted = 0.0

            def drain(frac):
                nonlocal qi, emitted
                emitted += frac
                while qi < len(queue) and qi < emitted:
                    queue[qi]()
                    qi += 1

            per_step = (0.7 if qc < 3 else 0.5) * len(queue) / n_steps
            for hp in range(2):
                po = ps.tile([128, 2, 512], F32, tag="po", name=f"po{qc}{hp}", bufs=1)
                for i in range(2):
                    # one start=True matmul zeroes all four qt accumulation
                    # regions of this bank (psum pending-zero is bank-wide)
                    nc.tensor.matmul(po[:, i, 0:260], zro_sb[0:1, 0:128],
                                     zro_sb[0:1, 0:260], start=True, stop=False,
                                     skip_group_check=True)
                for ki in range(n_ki):
                    j = ki - 4 * qc  # >= 0 on diagonal tiles
                    o_exp = max(0, 128 * j)
                    o_sc = min(o_exp, 256)  # f32r moving dim must be >= 256
                    sp = ps.tile([128, 2, 512], F32, tag="s",
                                 name=f"sp{qc}{hp}{ki}")
                    for i in range(2):
                        vp = 64 * i
                        nc.tensor.matmul(
                            sp[:, i, o_sc:512],
                            kT[vp:vp + 64, hp, ki * 128:(ki + 1) * 128],
                            qT[vp:vp + 64, hp, qc * 512 + o_sc:(qc + 1) * 512],
                            start=True, stop=True, tile_position=(vp, 0))
                    e = epool.tile([128, 2, 512], BF16, tag="e",
                                   name=f"e{qc}{hp}{ki}")
                    nc.scalar.activation(e[:, :, o_exp:512], sp[:, :, o_exp:512],
                                         EXP, scale=0.125)
                    if j >= 0:  # diagonal: mask the [128,128] triangle block
                        for i in range(2):
                            es = e[:, i, o_exp:o_exp + 128]
                            nc.vector.tensor_mul(es, es, cm_sb[:])
                    # AV transposed: po[q, 65] += E_block.T @ Vn
                    for i in range(2):
                        for qt in range(max(0, j), 4):
                            nc.tensor.matmul(
                                po[:, i, qt * 65:qt * 65 + 65],
                                e[:, i, qt * 128:(qt + 1) * 128],
                                vn[:, ki, 2 * hp + i, :],
                                start=False, stop=(ki == 4 * qc + qt),
                                skip_group_check=True)
                    drain(per_step)
                # normalize: per-partition recip of denominators, then scale
                rc = spool.tile([128, 2, 4], F32, tag="rc", name=f"rc{qc}{hp}")
                for i in range(2):
                    dn = po[:, i, 0:260].rearrange("p (qt c) -> p qt c", c=65)
                    with nc.allow_low_precision(reason="softmax recip"):
                        nc.vector.reciprocal(rc[:, i, :], dn[:, 0:4, 64:65])
                    for qt in range(4):
                        nc.vector.tensor_scalar_mul(
                            vst[:, qt, 2 * hp + i, :],
                            po[:, i, qt * 65:qt * 65 + 64],
                            rc[:, i, qt:qt + 1])
                drain(0.0)
            drain(len(queue))

        qk_tile(0, 0)
        qk_tile(0, 2)
        for j in range(4):
            v_tile(0, j)
        for c in range(SC):
            queue = []
            if c == 0:
                queue += [lambda mt=mt: qk_tile(0, mt) for mt in (1, 3)]
            else:
                queue += qkv_late(c)
            if c + 1 < SC:
                queue += qkv_early(c + 1)
            if c >= 1:
                queue += post_pieces(c - 1)
            attn_qc(c, queue)
        for piece in post_pieces(SC - 1):
            piece()
        if dbg:
            nc.sync.dma_start(d_vst, vst_tiles[0][:])
            nc.sync.dma_start(d_qT, qT[:])
            nc.sync.dma_start(d_kT, kT[:])
            nc.sync.dma_start(d_vn, vn[:])
            nc.sync.dma_start(d_vnT, vnT[:])

    if fix_waits:
        _fix_sync_waits(nc)
    return nc


def _get_nc():
    if "nc" not in _CACHE:
        _CACHE["nc"] = _build()
    return _CACHE["nc"]


def _dr_layout(xb):
    """[S, 1024] -> [128, 4, 2, S]: p=partition, kp=k-tile-pair, sl=slot."""
    return np.ascontiguousarray(
        xb.T.reshape(4, 2, 128, xb.shape[0]).transpose(2, 0, 1, 3))


def kernel(x, W_qkv, b_qkv, W_out, b_out):
    x = np.asarray(x, np.float32)
    W_qkv = np.asarray(W_qkv, np.float32)
    b_qkv = np.asarray(b_qkv, np.float32)
    W_out = np.asarray(W_out, np.float32)
    b_out = np.asarray(b_out, np.float32)

    nc = _get_nc()

    kk = np.arange(128)[:, None]
    qq = np.arange(128)[None, :]
    cmask = (kk <= qq).astype(BF)
    identb = np.eye(128, dtype=np.float32).astype(BF)
    vone = np.ones((128, KT, 4, 1), np.float32).astype(BF)

    in_maps = []
    for c in range(N_CORES):
        b, g = divmod(c, 4)
        heads = [4 * g + i for i in range(HL)]

        xb = x[b]                                        # [S, 1024]
        xr = _dr_layout(xb)
        x8 = xr.astype(E4)
        x8l = ((xr - x8.astype(np.float32)) * 8.0).astype(E4)
        xs8_a = (xr * 0.125).astype(E4)

        # qk weight m-tiles: mt0=q-hp0, mt1=q-hp1, mt2=k-hp0, mt3=k-hp1
        # out-col within tile = 64*i + dd  (i head-in-pair, dd hd index)
        wq = np.zeros((1024, 4, 128), np.float32)
        bqv = np.zeros((128, 4), np.float32)
        for mt in range(4):
            t, hp = divmod(mt, 2)       # t: 0=q, 1=k
            for i in range(2):
                h = heads[2 * hp + i]
                cols = h * 192 + 64 * t + np.arange(64)
                wq[:, mt, 64 * i:64 * i + 64] = W_qkv[:, cols]
                bqv[64 * i:64 * i + 64, mt] = b_qkv[cols]
        wq = wq.reshape(4, 2, 128, 4, 128).transpose(2, 0, 1, 3, 4)
        wq8 = wq.astype(E4)
        wq8l = ((wq - wq8.astype(np.float32)) * 8.0).astype(E4)
        wq8s = (wq * 0.125).astype(E4)

        # v weights: col = 64*h + dd
        wv = np.zeros((1024, 256), np.float32)
        bvv = np.zeros((4, 64), np.float32)
        for hh in range(4):
            cols = heads[hh] * 192 + 128 + np.arange(64)
            wv[:, 64 * hh:64 * hh + 64] = W_qkv[:, cols]
            bvv[hh] = b_qkv[cols]
        wv = wv.reshape(4, 2, 128, 256).transpose(2, 0, 1, 3)
        wv8 = wv.astype(E4)
        wv8l = ((wv - wv8.astype(np.float32)) * 8.0).astype(E4)
        wv8s = (wv * 0.125).astype(E4)
        bv2 = np.broadcast_to(bvv[None], (128, 4, 64))

        wo = W_out[g * 256:(g + 1) * 256, :]             # [256, 1024]
        wob = wo.reshape(2, 128, D).transpose(1, 0, 2).astype(BF)

        in_maps.append({
            "xq8": x8,
            "xq8l": x8l,
            "xs8": xs8_a,
            "zro": np.zeros((128, 384), np.float32),
            "wq8": np.ascontiguousarray(wq8),
            "wq8l": np.ascontiguousarray(wq8l),
            "wq8s": np.ascontiguousarray(wq8s),
            "wv8": np.ascontiguousarray(wv8),
            "wv8l": np.ascontiguousarray(wv8l),
            "wv8s": np.ascontiguousarray(wv8s),
            "woutb": np.ascontiguousarray(wob),
            "bq": np.ascontiguousarray(bqv),
            "bv": np.ascontiguousarray(bv2),
            "vone": vone,
            "cmask": np.ascontiguousarray(cmask),
            "identb": identb,
        })

    _CACHE["in_maps"] = in_maps
    res = bass_utils.run_bass_kernel_spmd(nc, in_maps, core_ids=list(range(N_CORES)))

    out = np.zeros((B, S, D), np.float32)
    for c in range(N_CORES):
        b = c // 4
        oT = np.asarray(res.results[c]["outT"]).astype(np.float32)
        out[b] += oT.transpose(1, 0, 2).reshape(D, S).T
    out += b_out
    return out



# revision 3
# speedup vs baseline: 1.0537x; 1.0537x over previous
"""Multi-head causal attention (B=2, S=2048, D=1024, H=16) on 8 TRN2 NeuronCores.

Sharding: core c handles batch b = c // 4 and local head group g = c % 4
(global heads 4g..4g+3).  Each core computes its heads' QKV projections,
causal attention, and a partial output projection; host sums the 4 partials
per batch and adds b_out.

v3 design (vs v2 at 129.2us):
  - Score-ahead pipelining: scores for ki+2 are emitted before AV(ki) in PE
    program order, so ACT (exp) runs back-to-back instead of ping-ponging
    with PE.  exp is the per-ki long pole (1024 els x 0.83ns vs PE 644ns).
  - Causal mask folded into the scores psum accumulation as a PE matmul:
    diag(-1e9) @ strict-upper-tri accumulated before the f32r score matmul.
    exp(-1.25e8) = 0, so the post-exp DVE mask multiply is gone.
  - Psum pending-zero folded into the first AV matmul of each bank
    (start=True zeroes the bank) -- the zro dummy matmuls are gone.
  - Startup: mt-major weight layout + per-mt weight DMAs + per-kp x DMAs,
    ordered by first use; chunk-0 hp0 QK projections run pass-major so the
    first scores (and exp) start as soon as pass-3 bytes land.
  - psum->sbuf copies (vnT, ou) moved from DVE to the idle GPSIMD engine.
  - Last chunk tail pipelined per qt: normalize/transpose/out-proj/DMA for
    qt fire as soon as its AV accumulation stops (ki = 12+qt).
  - Explicit drain-ordering (labels) replaces pacing-only correctness.
"""

from contextlib import ExitStack

import numpy as np
import ml_dtypes

import concourse.bass as bass
import concourse.mybir as mybir
import concourse.tile as tile
from concourse import bass_utils

F32 = mybir.dt.float32
F32R = mybir.dt.float32r
BF16 = mybir.dt.bfloat16
FP8 = mybir.dt.float8e4
EXP = mybir.ActivationFunctionType.Exp
COPY = mybir.ActivationFunctionType.Copy
DR = mybir.MatmulPerfMode.DoubleRow

E4 = ml_dtypes.float8_e4m3
BF = ml_dtypes.bfloat16

B, S, D, H = 2, 2048, 1024, 16
HD = D // H          # 64
HL = 4               # heads per core
N_CORES = 8
SC = S // 512        # 4 q-chunks of 512
KT = S // 128        # 16 k-tiles of 128
MTX = {0: 0, 2: 1, 1: 2, 3: 3}  # mt -> stored position (hp0 pair first)

_CACHE = {}
_PACE = [0.4, 0.6, 0.6, 0.4]


def _round_f32r(x: np.ndarray) -> np.ndarray:
    """Round f32 to fp32r (11-bit mantissa, RNE) on host."""
    u = np.ascontiguousarray(x, dtype=np.float32).view(np.uint32)
    frac = u & np.uint32(0x00000FFF)
    base = u & np.uint32(0xFFFFF000)
    bit = np.uint32(0x00000800)
    lsb = np.uint32(0x00001000)
    roundup = (frac > bit) | ((frac == bit) & ((u & lsb) != 0))
    return np.where(roundup, base + lsb, base).view(np.float32)


_NO_HOIST = {
    "AllEngineBarrier",
    "EventSemaphore",
    "UnconditionalBranch",
    "CompareAndBranch",
    "BranchHint",
    "IndirectBranch",
    "Halt",
    "Call",
    "OverlayCall",
    "NoOp",
}


def _fix_sync_waits(nc):
    """walrus codegen holds only one sync-wait per engine instruction; hoist
    excess waits onto same-engine NoOps inserted right before."""
    for fn in nc.m.functions:
        for blk in fn.blocks:
            insts = blk.instructions
            out = []
            changed = False
            for inst in insts:
                si = inst.sync_info
                if si is not None and inst.opcode not in _NO_HOIST:
                    waits = list(si.on_wait)
                    if len(waits) > 1:
                        for j, w in enumerate(waits[:-1]):
                            nop = mybir.InstNoOp(name=f"{inst.name}-wfix{j}")
                            nop.engine = inst.engine
                            nop.sync_info = mybir.SyncInfo(on_wait=[w], on_update=[])
                            out.append(nop)
                        inst.sync_info = mybir.SyncInfo(
                            on_wait=[waits[-1]], on_update=list(si.on_update)
                        )
                        changed = True
                out.append(inst)
            if changed:
                blk.instructions = out


class _Q:
    """Emission-time work queue with credit pacing + forced ordering."""

    def __init__(self):
        self.items = []      # (fn, label)
        self.qi = 0
        self.credit = 0.0

    def push(self, fn, label=None):
        self.items.append((fn, label))

    def remaining(self):
        return len(self.items) - self.qi

    def _emit_one(self):
        fn, _ = self.items[self.qi]
        self.qi += 1
        fn()

    def drain_frac(self, frac):
        self.credit += frac
        while self.qi < len(self.items) and self.qi < self.credit:
            self._emit_one()

    def drain_to(self, label):
        """Emit everything up to and including the piece tagged `label`."""
        done = any(lb == label for _, lb in self.items[: self.qi])
        if done:
            return
        while self.qi < len(self.items):
            lb = self.items[self.qi][1]
            self._emit_one()
            self.credit = max(self.credit, self.qi)
            if lb == label:
                return
        raise KeyError(f"label {label} not found in queue")

    def flush(self):
        while self.qi < len(self.items):
            self._emit_one()
        self.credit = self.qi


def _build(fix_waits=True, dbg=False):
    nc = bass.Bass("TRN2", target_bir_lowering=False, debug=False,
                   num_devices=N_CORES)
    if dbg:
        d_qT = nc.dram_tensor("d_qT", [128, 2, S], F32R, kind="ExternalOutput").ap()
        d_kT = nc.dram_tensor("d_kT", [128, 2, S], F32R, kind="ExternalOutput").ap()
        d_vn = nc.dram_tensor("d_vn", [128, KT, 4, 65], BF16,
                              kind="ExternalOutput").ap()
        d_vst = nc.dram_tensor("d_vst", [128, 4, 4, 64], BF16,
                               kind="ExternalOutput").ap()
        d_vnT = nc.dram_tensor("d_vnT", [128, 2, S], BF16,
                               kind="ExternalOutput").ap()

    # x in fp8 hi / lo*8 / /8 copies, [128, kp, sl, S]
    xq8 = nc.dram_tensor("xq8", [128, 4, 2, S], FP8, kind="ExternalInput").ap()
    xq8l = nc.dram_tensor("xq8l", [128, 4, 2, S], FP8, kind="ExternalInput").ap()
    xs8 = nc.dram_tensor("xs8", [128, 4, 2, S], FP8, kind="ExternalInput").ap()
    # qk weights mt-major: [128, mt, kp, sl, 128]
    wq8 = nc.dram_tensor("wq8", [128, 4, 4, 2, 128], FP8, kind="ExternalInput").ap()
    wq8l = nc.dram_tensor("wq8l", [128, 4, 4, 2, 128], FP8, kind="ExternalInput").ap()
    wq8s = nc.dram_tensor("wq8s", [128, 4, 4, 2, 128], FP8, kind="ExternalInput").ap()
    wv8 = nc.dram_tensor("wv8", [128, 4, 2, 256], FP8, kind="ExternalInput").ap()
    wv8l = nc.dram_tensor("wv8l", [128, 4, 2, 256], FP8, kind="ExternalInput").ap()
    wv8s = nc.dram_tensor("wv8s", [128, 4, 2, 256], FP8, kind="ExternalInput").ap()
    woutb = nc.dram_tensor("woutb", [128, 2, D], BF16, kind="ExternalInput").ap()
    bq = nc.dram_tensor("bq", [128, 4], F32, kind="ExternalInput").ap()
    bv = nc.dram_tensor("bv", [128, 4, 64], F32, kind="ExternalInput").ap()
    dmsk = nc.dram_tensor("dmsk", [128, 128], BF16, kind="ExternalInput").ap()
    utri = nc.dram_tensor("utri", [128, 128], BF16, kind="ExternalInput").ap()
    identb = nc.dram_tensor("identb", [128, 128], BF16, kind="ExternalInput").ap()
    outT = nc.dram_tensor("outT", [128, 8, S], BF16, kind="ExternalOutput").ap()

    with tile.TileContext(nc) as tc, ExitStack() as ctx:
        persist = ctx.enter_context(tc.tile_pool(name="persist", bufs=1))
        xpool = ctx.enter_context(tc.tile_pool(name="xp", bufs=3))
        epool = ctx.enter_context(tc.tile_pool(name="ep", bufs=8))
        spool = ctx.enter_context(tc.tile_pool(name="stp", bufs=3))
        opool = ctx.enter_context(tc.tile_pool(name="op", bufs=6))
        # psum (8 banks): s 2x2-bank (sp / startup pq), po 1x2-bank,
        # q1 2x1-bank (pv/pq/pu/tr churn)
        ps = ctx.enter_context(tc.tile_pool(name="ps", bufs=2, space="PSUM"))

        wq_sb = persist.tile([128, 4, 4, 2, 128], FP8, tag="wq")
        wql_sb = persist.tile([128, 4, 4, 2, 128], FP8, tag="wql")
        wqs_sb = persist.tile([128, 4, 4, 2, 128], FP8, tag="wqs")
        wv_sb = persist.tile([128, 4, 2, 256], FP8, tag="wv")
        wvl_sb = persist.tile([128, 4, 2, 256], FP8, tag="wvl")
        wvs_sb = persist.tile([128, 4, 2, 256], FP8, tag="wvs")
        wo_sb = persist.tile([128, 2, D], BF16, tag="wo")
        bq_sb = persist.tile([128, 4], F32, tag="bq")
        bv_sb = persist.tile([128, 4, 64], F32, tag="bv")
        dm_sb = persist.tile([128, 128], BF16, tag="dm")
        ut_sb = persist.tile([128, 128], BF16, tag="ut")
        id_sb = persist.tile([128, 128], BF16, tag="id")
        qT = persist.tile([128, 2, S], F32R, tag="qT")
        kT = persist.tile([128, 2, S], F32R, tag="kT")
        vn = persist.tile([128, KT, 4, 65], BF16, tag="vn")
        vnT = persist.tile([128, 2, S], BF16, tag="vnT")

        # ---- startup DMAs, ordered by first use ----
        # weight mt axis is stored in order [0, 2, 1, 3] so the hp0 pair
        # (mt 0 and 2) is one contiguous 256KB DMA.
        xc0 = xpool.tile([128, 4, 2, 512], FP8, tag="xc", name="xc0")
        xl0 = xpool.tile([128, 4, 2, 512], FP8, tag="xl", name="xl0")
        xs0 = xpool.tile([128, 4, 2, 512], FP8, tag="xs", name="xs0")
        # pass 1: wq mt0/mt2 + xc0, split fine for first-byte latency
        nc.scalar.dma_start(wq_sb[:, 0:1], wq8[:, 0:1])
        nc.sync.dma_start(xc0[:, 0:2], xq8[:, 0:2, :, 0:512])
        nc.scalar.dma_start(wq_sb[:, 1:2], wq8[:, 1:2])
        nc.sync.dma_start(xc0[:, 2:4], xq8[:, 2:4, :, 0:512])
        nc.scalar.dma_start(wv_sb[:], wv8)
        # pass 2: wql mt0/mt2 + xs0
        nc.scalar.dma_start(wql_sb[:, 0:2], wq8l[:, 0:2])
        nc.sync.dma_start(xs0[:], xs8[:, :, :, 0:512])
        nc.scalar.dma_start(wvl_sb[:], wv8l)
        # pass 3: wqs mt0/mt2 + xl0
        nc.scalar.dma_start(wqs_sb[:, 0:2], wq8s[:, 0:2])
        nc.sync.dma_start(xl0[:], xq8l[:, :, :, 0:512])
        nc.sync.dma_start(bq_sb[:], bq)
        nc.sync.dma_start(dm_sb[:], dmsk)
        nc.sync.dma_start(ut_sb[:], utri)
        nc.scalar.dma_start(wvs_sb[:], wv8s)
        nc.sync.dma_start(bv_sb[:], bv)
        # hp1 qk weights (mt 1 and 3 = stored positions 2:4)
        nc.scalar.dma_start(wq_sb[:, 2:4], wq8[:, 2:4])
        nc.scalar.dma_start(wql_sb[:, 2:4], wq8l[:, 2:4])
        nc.scalar.dma_start(wqs_sb[:, 2:4], wq8s[:, 2:4])
        nc.scalar.dma_start(id_sb[:], identb)
        nc.scalar.dma_start(wo_sb[:], woutb)
        # ones column of vn (softmax denominators) via memset, not DMA
        nc.vector.memset(vn[:, :, :, 64:65], 1.0)
        xtiles = {0: (xc0, xl0, xs0)}

        def qkv_dma(qc):
            qs = slice(qc * 512, (qc + 1) * 512)
            xc = xpool.tile([128, 4, 2, 512], FP8, tag="xc", name=f"xc{qc}")
            xl = xpool.tile([128, 4, 2, 512], FP8, tag="xl", name=f"xl{qc}")
            xs = xpool.tile([128, 4, 2, 512], FP8, tag="xs", name=f"xs{qc}")
            nc.sync.dma_start(xc[:], xq8[:, :, :, qs])
            nc.sync.dma_start(xl[:], xq8l[:, :, :, qs])
            nc.sync.dma_start(xs[:], xs8[:, :, :, qs])
            xtiles[qc] = (xc, xl, xs)

        pq_tiles = {}

        def qk_pass(qc, mt, p, tag="q1"):
            """One error-compensation pass (4 DR matmuls) of a q/k tile."""
            xc, xl, xs = xtiles[qc]
            if p == 0:
                pq_tiles[(qc, mt)] = ps.tile([128, 512], F32, tag=tag,
                                             name=f"pq{qc}{mt}")
            pq = pq_tiles[(qc, mt)]
            wsb, xsb = [(wq_sb, xc), (wql_sb, xs), (wqs_sb, xl)][p]
            mtx = MTX[mt]
            for kp in range(4):
                nc.tensor.matmul(
                    pq[:], wsb[:, mtx, kp, :, :], xsb[:, kp, :, :],
                    start=(p == 0 and kp == 0), stop=(p == 2 and kp == 3),
                    perf_mode=DR)

        def qk_bias(qc, mt):
            qs = slice(qc * 512, (qc + 1) * 512)
            pq = pq_tiles.pop((qc, mt))
            dst = (qT if mt < 2 else kT)[:, mt % 2, qs]
            nc.vector.tensor_scalar_add(dst, pq[:], bq_sb[:, mt:mt + 1])

        def qk_tile(qc, mt):
            for p in range(3):
                qk_pass(qc, mt, p)
            qk_bias(qc, mt)

        pv_tiles = {}

        def v_pass(qc, j, p, tag="q1"):
            xc, xl, xs = xtiles[qc]
            if p == 0:
                pv_tiles[(qc, j)] = ps.tile([128, 256], F32, tag=tag,
                                            name=f"pv{qc}{j}")
            pv = pv_tiles[(qc, j)]
            wsb, xsb = [(wv_sb, xc), (wvl_sb, xs), (wvs_sb, xl)][p]
            for kp in range(4):
                nc.tensor.matmul(
                    pv[:], xsb[:, kp, :, j * 128:(j + 1) * 128],
                    wsb[:, kp, :, :],
                    start=(p == 0 and kp == 0), stop=(p == 2 and kp == 3),
                    perf_mode=DR)

        def v_bias(qc, j):
            st = 4 * qc + j
            pv = pv_tiles.pop((qc, j))
            nc.vector.tensor_add(
                vn[:, st, :, 0:64],
                pv[:].rearrange("p (h d) -> p h d", h=4),
                bv_sb[:])

        def v_tile(qc, j):
            st = 4 * qc + j
            xc, xl, xs = xtiles[qc]
            pv = ps.tile([128, 256], F32, tag="q1", name=f"pv{qc}{j}")
            passes = [(wv_sb, xc), (wvl_sb, xs), (wvs_sb, xl)]
            i = 0
            for wsb, xsb in passes:
                for kp in range(4):
                    nc.tensor.matmul(
                        pv[:], xsb[:, kp, :, j * 128:(j + 1) * 128],
                        wsb[:, kp, :, :],
                        start=(i == 0), stop=(i == 11), perf_mode=DR)
                    i += 1
            nc.vector.tensor_add(
                vn[:, st, :, 0:64],
                pv[:].rearrange("p (h d) -> p h d", h=4),
                bv_sb[:])

        vst_tiles = {}

        def tr_piece(qc, qt, dhs=(0, 1), copy_eng=None):
            vst = vst_tiles[qc]
            for dh in dhs:
                ptr = ps.tile([128, 128], BF16, tag="q1", name=f"tr{qc}{qt}{dh}")
                nc.tensor.matmul(ptr[:], vst[:, qt, 2 * dh:2 * dh + 2, :],
                                 id_sb[:], is_transpose=True)
                eng = copy_eng or nc.vector
                dst = vnT[:, dh, qc * 512 + qt * 128:qc * 512 + (qt + 1) * 128]
                if eng is nc.scalar:
                    eng.copy(dst, ptr[:])
                else:
                    eng.tensor_copy(dst, ptr[:])

        def op_pair(qc, mp, tags=("q1", "q1"), engs=None, split_dma=False):
            """Out-proj for heads-pair mp (m = 2mp, 2mp+1): 4 matmuls, two
            psum->sbuf copies, ONE fused output DMA (HWDGE is a single
            global device at ~630ns per DMA, so fewer DMAs win)."""
            qs = slice(qc * 512, (qc + 1) * 512)
            ou = opool.tile([128, 2, 512], BF16, tag="ou", name=f"ou{qc}{mp}")
            for j, m in enumerate((2 * mp, 2 * mp + 1)):
                pu = ps.tile([128, 512], F32, tag=tags[j], name=f"pu{qc}{m}")
                for t in range(2):
                    nc.tensor.matmul(pu[:], wo_sb[:, t, m * 128:(m + 1) * 128],
                                     vnT[:, t, qs], start=(t == 0), stop=(t == 1))
                eng = engs[j] if engs else nc.vector
                if eng is nc.scalar:
                    eng.copy(ou[:, j], pu[:])
                else:
                    eng.tensor_copy(ou[:, j], pu[:])
                if split_dma:
                    (nc.sync if j == 0 else nc.scalar).dma_start(
                        outT[:, m, qs], ou[:, j])
            if not split_dma:
                (nc.sync if mp % 2 == 0 else nc.scalar).dma_start(
                    outT[:, 2 * mp:2 * mp + 2, qs], ou[:])

        def op_tail(qc, mp):
            """Tail out-proj pair: pu psum uses the q1 and (now free) s tags;
            copies round-robin DVE/ACT/GPSIMD to pipeline behind PE."""
            engs = [(nc.vector, nc.scalar), (nc.vector, nc.scalar),
                    (nc.scalar, nc.vector), (nc.vector, nc.scalar)][mp]
            op_pair(qc, mp, tags=("q1", "s"), engs=engs, split_dma=(mp == 3))

        queue = _Q()

        def push_qkv_late(c):
            for j in (2, 3):
                queue.push(lambda c=c, j=j: v_tile(c, j), f"v{c}{j}")
            for mt in (1, 3):
                queue.push(lambda c=c, mt=mt: qk_tile(c, mt), f"qk{c}{mt}")

        def push_qkv_early(c):
            queue.push(lambda c=c: qkv_dma(c), f"dma{c}")
            for mt in (0, 2):
                queue.push(lambda c=c, mt=mt: qk_tile(c, mt), f"qk{c}{mt}")
            for j in (0, 1):
                queue.push(lambda c=c, j=j: v_tile(c, j), f"v{c}{j}")

        def push_post(c):
            for qt in range(4):
                queue.push(lambda c=c, qt=qt: tr_piece(c, qt), f"tr{c}{qt}")
            for mp in range(4):
                queue.push(lambda c=c, mp=mp: op_pair(c, mp), f"op{c}{mp}")

        def attn_group(qc, hp, inline=None, per_step=0.0, tail=False,
                       need=None):
            """Attention for group (qc, hp) with score-ahead pipelining.

            inline: optional dict ki -> [fn] of pieces emitted right before
            AV(ki) (used for chunk 0's v tiles).  need: dict ki -> queue
            label that must be emitted before AV(ki) (vn dependencies).
            tail=True pipelines the last chunk's normalize/transpose/
            out-proj per qt.
            """
            vst = vst_tiles[qc]
            n_ki = 4 * qc + 4

            def sc(ki):
                j = ki - 4 * qc
                o_exp = max(0, 128 * j)
                o_sc = min(o_exp, 256)  # f32r moving dim must be >= 256
                sp = ps.tile([128, 2, 512], F32, tag="s", name=f"sp{qc}{hp}{ki}")
                for i in range(2):
                    vp = 64 * i
                    if j >= 0:
                        # causal mask: psum[k, q] -= 1e9 * [k > q] on the
                        # diagonal block, via diag(-1e9) @ strict-upper-tri
                        nc.tensor.matmul(
                            sp[:, i, o_exp:o_exp + 128], dm_sb[:], ut_sb[:],
                            start=True, stop=False, skip_group_check=True)
                    nc.tensor.matmul(
                        sp[:, i, o_sc:512],
                        kT[vp:vp + 64, hp, ki * 128:(ki + 1) * 128],
                        qT[vp:vp + 64, hp, qc * 512 + o_sc:(qc + 1) * 512],
                        start=(j < 0), stop=True, tile_position=(vp, 0),
                        skip_group_check=True)
                e = epool.tile([128, 2, 512], BF16, tag="e", name=f"e{qc}{hp}{ki}")
                nc.scalar.activation(e[:, :, o_exp:512], sp[:, :, o_exp:512],
                                     EXP, scale=0.125)
                return e

            def av(ki, e):
                j = ki - 4 * qc
                for i in range(2):
                    for qt in range(max(0, j), 4):
                        nc.tensor.matmul(
                            po[:, i, qt * 65:qt * 65 + 65],
                            e[:, i, qt * 128:(qt + 1) * 128],
                            vn[:, ki, 2 * hp + i, :],
                            start=(ki == 0 and qt == 0),
                            stop=(ki == 4 * qc + qt),
                            skip_group_check=True)

            def norm_qt(qt):
                for i in range(2):
                    dn = po[:, i, 0:260].rearrange("p (qt c) -> p qt c", c=65)
                    with nc.allow_low_precision(reason="softmax recip"):
                        nc.vector.reciprocal(rc[:, i, qt:qt + 1],
                                             dn[:, qt, 64:65])
                    nc.vector.tensor_scalar_mul(
                        vst[:, qt, 2 * hp + i, :],
                        po[:, i, qt * 65:qt * 65 + 64],
                        rc[:, i, qt:qt + 1])

            def norm_all():
                for i in range(2):
                    dn = po[:, i, 0:260].rearrange("p (qt c) -> p qt c", c=65)
                    with nc.allow_low_precision(reason="softmax recip"):
                        nc.vector.reciprocal(rc[:, i, :], dn[:, 0:4, 64:65])
                    for qt in range(4):
                        nc.vector.tensor_scalar_mul(
                            vst[:, qt, 2 * hp + i, :],
                            po[:, i, qt * 65:qt * 65 + 64],
                            rc[:, i, qt:qt + 1])

            po = ps.tile([128, 2, 512], F32, tag="po", name=f"po{qc}{hp}", bufs=1)
            rc = spool.tile([128, 2, 4], F32, tag="rc", name=f"rc{qc}{hp}")
            es = {}
            es[0] = sc(0)
            if n_ki > 1:
                es[1] = sc(1)
            for ki in range(n_ki):
                if inline:
                    for fn in inline.get(ki, ()):
                        fn()
                if need and ki in need:
                    queue.drain_to(need[ki])
                av(ki, es.pop(ki))
                if ki + 2 < n_ki:
                    es[ki + 2] = sc(ki + 2)
                if tail and ki >= n_ki - 4:
                    qt = ki - (n_ki - 4)
                    norm_qt(qt)
                    tr_piece(qc, qt, dhs=(0,), copy_eng=nc.scalar)
                    tr_piece(qc, qt, dhs=(1,), copy_eng=nc.vector)
                else:
                    queue.drain_frac(per_step)
            if tail:
                for mp in range(4):
                    op_tail(qc, mp)
            else:
                norm_all()

        # ---- chunk 0: hp0 qk + v0/v1 pass-major so PE has work while
        # pass-2/3 bytes stream in and first scores start ASAP ----
        for p in range(3):
            qk_pass(0, 0, p, tag="s")
            qk_pass(0, 2, p, tag="s")
            v_pass(0, 0, p)
            v_pass(0, 1, p)
        qk_bias(0, 0)
        qk_bias(0, 2)
        v_bias(0, 0)
        v_bias(0, 1)

        vst_tiles[0] = spool.tile([128, 4, 4, 64], BF16, tag="vst", name="vs0")
        # chunk 0's v tiles run inline between AVs; only qk hp1 is queued
        for mt in (1, 3):
            queue.push(lambda mt=mt: qk_tile(0, mt), f"qk0{mt}")
        push_qkv_early(1)
        inline0 = {ki: [lambda ki=ki: v_tile(0, ki)] for ki in (2, 3)}
        attn_group(0, 0, inline=inline0,
                   per_step=_PACE[0] * queue.remaining() / 4)

        for qc in range(SC):
            n_ki = 4 * qc + 4
            if qc > 0:
                vst_tiles[qc] = spool.tile([128, 4, 4, 64], BF16, tag="vst",
                                           name=f"vs{qc}")
                push_qkv_late(qc)
                if qc + 1 < SC:
                    push_qkv_early(qc + 1)
                push_post(qc - 1)
                # scores need this chunk's qT/kT hp0; AV(ki) needs vn[ki]
                queue.drain_to(f"qk{qc}2")
                need = {4 * qc + j: f"v{qc}{j}" for j in range(4)}
                f0 = _PACE[3] if qc == SC - 1 else _PACE[1]
                attn_group(qc, 0, need=need,
                           per_step=f0 * queue.remaining() / n_ki)
            # hp1 needs this chunk's mt=1,3 projections emitted first
            queue.drain_to(f"qk{qc}3")
            if qc < SC - 1:
                attn_group(qc, 1, per_step=_PACE[2] * queue.remaining() / n_ki)
            else:
                attn_group(qc, 1, tail=True,
                           per_step=queue.remaining() / (n_ki - 4))

        queue.flush()
        if dbg:
            nc.sync.dma_start(d_vst, vst_tiles[0][:])
            nc.sync.dma_start(d_qT, qT[:])
            nc.sync.dma_start(d_kT, kT[:])
            nc.sync.dma_start(d_vn, vn[:])
            nc.sync.dma_start(d_vnT, vnT[:])

    if fix_waits:
        _fix_sync_waits(nc)
    return nc


def _get_nc():
    if "nc" not in _CACHE:
        _CACHE["nc"] = _build()
    return _CACHE["nc"]


def _dr_layout(xb):
    """[S, 1024] -> [128, 4, 2, S]: p=partition, kp=k-tile-pair, sl=slot."""
    return np.ascontiguousarray(
        xb.T.reshape(4, 2, 128, xb.shape[0]).transpose(2, 0, 1, 3))


def kernel(x, W_qkv, b_qkv, W_out, b_out):
    x = np.asarray(x, np.float32)
    W_qkv = np.asarray(W_qkv, np.float32)
    b_qkv = np.asarray(b_qkv, np.float32)
    W_out = np.asarray(W_out, np.float32)
    b_out = np.asarray(b_out, np.float32)

    nc = _get_nc()

    kk = np.arange(128)[:, None]
    qq = np.arange(128)[None, :]
    dmask = (-1e9 * np.eye(128, dtype=np.float32)).astype(BF)
    utri = (kk > qq).astype(BF)      # [r, q] = 1 where r > q
    identb = np.eye(128, dtype=np.float32).astype(BF)

    in_maps = []
    for c in range(N_CORES):
        b, g = divmod(c, 4)
        heads = [4 * g + i for i in range(HL)]

        xb = x[b]                                        # [S, 1024]
        xr = _dr_layout(xb)
        x8 = xr.astype(E4)
        x8l = ((xr - x8.astype(np.float32)) * 8.0).astype(E4)
        xs8_a = (xr * 0.125).astype(E4)

        # qk weight m-tiles: mt0=q-hp0, mt1=q-hp1, mt2=k-hp0, mt3=k-hp1
        # out-col within tile = 64*i + dd  (i head-in-pair, dd hd index)
        wq = np.zeros((1024, 4, 128), np.float32)
        bqv = np.zeros((128, 4), np.float32)
        for mt in range(4):
            t, hp = divmod(mt, 2)       # t: 0=q, 1=k
            for i in range(2):
                h = heads[2 * hp + i]
                cols = h * 192 + 64 * t + np.arange(64)
                wq[:, mt, 64 * i:64 * i + 64] = W_qkv[:, cols]
                bqv[64 * i:64 * i + 64, mt] = b_qkv[cols]
        # mt axis stored as [0,2,1,3]; [1024, mt, 128] -> [128(p), mt, kp, sl, 128]
        wq = wq[:, [0, 2, 1, 3], :]
        wq = wq.reshape(4, 2, 128, 4, 128).transpose(2, 3, 0, 1, 4)
        wq8 = wq.astype(E4)
        wq8l = ((wq - wq8.astype(np.float32)) * 8.0).astype(E4)
        wq8s = (wq * 0.125).astype(E4)

        # v weights: col = 64*h + dd
        wv = np.zeros((1024, 256), np.float32)
        bvv = np.zeros((4, 64), np.float32)
        for hh in range(4):
            cols = heads[hh] * 192 + 128 + np.arange(64)
            wv[:, 64 * hh:64 * hh + 64] = W_qkv[:, cols]
            bvv[hh] = b_qkv[cols]
        wv = wv.reshape(4, 2, 128, 256).transpose(2, 0, 1, 3)
        wv8 = wv.astype(E4)
        wv8l = ((wv - wv8.astype(np.float32)) * 8.0).astype(E4)
        wv8s = (wv * 0.125).astype(E4)
        bv2 = np.broadcast_to(bvv[None], (128, 4, 64))

        wo = W_out[g * 256:(g + 1) * 256, :]             # [256, 1024]
        wob = wo.reshape(2, 128, D).transpose(1, 0, 2).astype(BF)

        in_maps.append({
            "xq8": x8,
            "xq8l": x8l,
            "xs8": xs8_a,
            "wq8": np.ascontiguousarray(wq8),
            "wq8l": np.ascontiguousarray(wq8l),
            "wq8s": np.ascontiguousarray(wq8s),
            "wv8": np.ascontiguousarray(wv8),
            "wv8l": np.ascontiguousarray(wv8l),
            "wv8s": np.ascontiguousarray(wv8s),
            "woutb": np.ascontiguousarray(wob),
            "bq": np.ascontiguousarray(bqv),
            "bv": np.ascontiguousarray(bv2),
            "dmsk": np.ascontiguousarray(dmask),
            "utri": np.ascontiguousarray(utri),
            "identb": identb,
        })

    _CACHE["in_maps"] = in_maps
    res = bass_utils.run_bass_kernel_spmd(nc, in_maps, core_ids=list(range(N_CORES)))

    out = np.zeros((B, S, D), np.float32)
    for c in range(N_CORES):
        b = c // 4
        oT = np.asarray(res.results[c]["outT"]).astype(np.float32)
        out[b] += oT.transpose(1, 0, 2).reshape(D, S).T
    out += b_out
    return out


# revision 4
# speedup vs baseline: 1.0583x; 1.0044x over previous
"""Multi-head causal attention (B=2, S=2048, D=1024, H=16) on 8 TRN2 NeuronCores.

Sharding: core c handles batch b = c // 4 and local head group g = c % 4
(global heads 4g..4g+3).  Each core computes its heads' QKV projections,
causal attention, and a partial output projection; host sums the 4 partials
per batch and adds b_out.

v3 design, 122.1us TimelineSim (v2 baseline 129.2us):
  - Score-ahead pipelining: scores for ki+2 are emitted before AV(ki) in PE
    program order, so ACT (exp) runs back-to-back instead of ping-ponging
    with PE.  exp is the per-ki long pole (1024 els x 0.83ns vs PE 644ns).
  - Group-boundary pre-emit: the next (qc, hp) group's first two
    score+exp tiles are emitted inside the current group's tail (next_hook)
    so ACT has no bubble across hp/chunk transitions.
  - Causal mask folded into the scores psum accumulation as a PE matmul:
    diag(-1e9) @ strict-upper-tri accumulated before the f32r score matmul.
    exp(-1.25e8) = 0, so the post-exp DVE mask multiply is gone.
  - Psum pending-zero folded into the first AV matmul of each bank
    (start=True zeroes the bank) -- the zro dummy matmuls are gone.
  - Startup: weight mt axis stored [0,2,1,3] so hp0 slices are single DMAs,
    DMAs ordered by first use (few and large: HWDGE is a single global
    ~630ns/DMA device); chunk-0 hp0 QK + v0/v1 projections run pass-major
    so PE has work while pass-2/3 bytes stream in.
  - Output DMAs fused per head-pair (one [128,2,512] DMA per two m tiles).
  - Last chunk tail: per-qt normalize/transpose as each AV accumulation
    stops (ki = 12+qt); out-proj pairs use the freed s-tag psum slots for
    ring depth 4 with copies round-robin DVE/ACT.
  - Explicit drain-ordering (labels) replaces pacing-only correctness.
  - NOTE: GPSIMD cannot access PSUM on TRN2 (BIR verifier) -- all
    psum->sbuf moves must be on DVE or ACT.
"""

from contextlib import ExitStack

import numpy as np
import ml_dtypes

import concourse.bass as bass
import concourse.mybir as mybir
import concourse.tile as tile
from concourse import bass_utils

F32 = mybir.dt.float32
F32R = mybir.dt.float32r
BF16 = mybir.dt.bfloat16
FP8 = mybir.dt.float8e4
EXP = mybir.ActivationFunctionType.Exp
COPY = mybir.ActivationFunctionType.Copy
DR = mybir.MatmulPerfMode.DoubleRow

E4 = ml_dtypes.float8_e4m3
BF = ml_dtypes.bfloat16

B, S, D, H = 2, 2048, 1024, 16
HD = D // H          # 64
HL = 4               # heads per core
N_CORES = 8
SC = S // 512        # 4 q-chunks of 512
KT = S // 128        # 16 k-tiles of 128
MTX = {0: 0, 2: 1, 1: 2, 3: 3}  # mt -> stored position (hp0 pair first)

_CACHE = {}
_PACE = [0.4, 0.6, 0.6, 0.4]
_HOOKLAG = 2


def _round_f32r(x: np.ndarray) -> np.ndarray:
    """Round f32 to fp32r (11-bit mantissa, RNE) on host."""
    u = np.ascontiguousarray(x, dtype=np.float32).view(np.uint32)
    frac = u & np.uint32(0x00000FFF)
    base = u & np.uint32(0xFFFFF000)
    bit = np.uint32(0x00000800)
    lsb = np.uint32(0x00001000)
    roundup = (frac > bit) | ((frac == bit) & ((u & lsb) != 0))
    return np.where(roundup, base + lsb, base).view(np.float32)


_NO_HOIST = {
    "AllEngineBarrier",
    "EventSemaphore",
    "UnconditionalBranch",
    "CompareAndBranch",
    "BranchHint",
    "IndirectBranch",
    "Halt",
    "Call",
    "OverlayCall",
    "NoOp",
}


def _fix_sync_waits(nc):
    """walrus codegen holds only one sync-wait per engine instruction; hoist
    excess waits onto same-engine NoOps inserted right before."""
    for fn in nc.m.functions:
        for blk in fn.blocks:
            insts = blk.instructions
            out = []
            changed = False
            for inst in insts:
                si = inst.sync_info
                if si is not None and inst.opcode not in _NO_HOIST:
                    waits = list(si.on_wait)
                    if len(waits) > 1:
                        for j, w in enumerate(waits[:-1]):
                            nop = mybir.InstNoOp(name=f"{inst.name}-wfix{j}")
                            nop.engine = inst.engine
                            nop.sync_info = mybir.SyncInfo(on_wait=[w], on_update=[])
                            out.append(nop)
                        inst.sync_info = mybir.SyncInfo(
                            on_wait=[waits[-1]], on_update=list(si.on_update)
                        )
                        changed = True
                out.append(inst)
            if changed:
                blk.instructions = out


class _Q:
    """Emission-time work queue with credit pacing + forced ordering."""

    def __init__(self):
        self.items = []      # (fn, label)
        self.qi = 0
        self.credit = 0.0

    def push(self, fn, label=None):
        self.items.append((fn, label))

    def remaining(self):
        return len(self.items) - self.qi

    def _emit_one(self):
        fn, _ = self.items[self.qi]
        self.qi += 1
        fn()

    def drain_frac(self, frac):
        self.credit += frac
        while self.qi < len(self.items) and self.qi < self.credit:
            self._emit_one()

    def drain_to(self, label):
        """Emit everything up to and including the piece tagged `label`."""
        done = any(lb == label for _, lb in self.items[: self.qi])
        if done:
            return
        while self.qi < len(self.items):
            lb = self.items[self.qi][1]
            self._emit_one()
            self.credit = max(self.credit, self.qi)
            if lb == label:
                return
        raise KeyError(f"label {label} not found in queue")

    def flush(self):
        while self.qi < len(self.items):
            self._emit_one()
        self.credit = self.qi


def _build(fix_waits=True, dbg=False):
    nc = bass.Bass("TRN2", target_bir_lowering=False, debug=False,
                   num_devices=N_CORES)
    if dbg:
        d_qT = nc.dram_tensor("d_qT", [128, 2, S], F32R, kind="ExternalOutput").ap()
        d_kT = nc.dram_tensor("d_kT", [128, 2, S], F32R, kind="ExternalOutput").ap()
        d_vn = nc.dram_tensor("d_vn", [128, KT, 4, 65], BF16,
                              kind="ExternalOutput").ap()
        d_vst = nc.dram_tensor("d_vst", [128, 4, 4, 64], BF16,
                               kind="ExternalOutput").ap()
        d_vnT = nc.dram_tensor("d_vnT", [128, 2, S], BF16,
                               kind="ExternalOutput").ap()

    # x in fp8 hi / lo*8 / /8 copies, [128, kp, sl, S]
    xq8 = nc.dram_tensor("xq8", [128, 4, 2, S], FP8, kind="ExternalInput").ap()
    xq8l = nc.dram_tensor("xq8l", [128, 4, 2, S], FP8, kind="ExternalInput").ap()
    xs8 = nc.dram_tensor("xs8", [128, 4, 2, S], FP8, kind="ExternalInput").ap()
    # qk weights mt-major: [128, mt, kp, sl, 128]
    wq8 = nc.dram_tensor("wq8", [128, 4, 4, 2, 128], FP8, kind="ExternalInput").ap()
    wq8l = nc.dram_tensor("wq8l", [128, 4, 4, 2, 128], FP8, kind="ExternalInput").ap()
    wq8s = nc.dram_tensor("wq8s", [128, 4, 4, 2, 128], FP8, kind="ExternalInput").ap()
    wv8 = nc.dram_tensor("wv8", [128, 4, 2, 256], FP8, kind="ExternalInput").ap()
    wv8l = nc.dram_tensor("wv8l", [128, 4, 2, 256], FP8, kind="ExternalInput").ap()
    wv8s = nc.dram_tensor("wv8s", [128, 4, 2, 256], FP8, kind="ExternalInput").ap()
    woutb = nc.dram_tensor("woutb", [128, 2, D], BF16, kind="ExternalInput").ap()
    bq = nc.dram_tensor("bq", [128, 4], F32, kind="ExternalInput").ap()
    bv = nc.dram_tensor("bv", [128, 4, 64], F32, kind="ExternalInput").ap()
    dmsk = nc.dram_tensor("dmsk", [128, 128], BF16, kind="ExternalInput").ap()
    utri = nc.dram_tensor("utri", [128, 128], BF16, kind="ExternalInput").ap()
    identb = nc.dram_tensor("identb", [128, 128], BF16, kind="ExternalInput").ap()
    outT = nc.dram_tensor("outT", [128, 8, S], BF16, kind="ExternalOutput").ap()

    with tile.TileContext(nc) as tc, ExitStack() as ctx:
        persist = ctx.enter_context(tc.tile_pool(name="persist", bufs=1))
        xpool = ctx.enter_context(tc.tile_pool(name="xp", bufs=3))
        epool = ctx.enter_context(tc.tile_pool(name="ep", bufs=8))
        spool = ctx.enter_context(tc.tile_pool(name="stp", bufs=3))
        opool = ctx.enter_context(tc.tile_pool(name="op", bufs=6))
        # psum (8 banks): s 2x2-bank (sp / startup pq), po 1x2-bank,
        # q1 2x1-bank (pv/pq/pu/tr churn)
        ps = ctx.enter_context(tc.tile_pool(name="ps", bufs=2, space="PSUM"))

        wq_sb = persist.tile([128, 4, 4, 2, 128], FP8, tag="wq")
        wql_sb = persist.tile([128, 4, 4, 2, 128], FP8, tag="wql")
        wqs_sb = persist.tile([128, 4, 4, 2, 128], FP8, tag="wqs")
        wv_sb = persist.tile([128, 4, 2, 256], FP8, tag="wv")
        wvl_sb = persist.tile([128, 4, 2, 256], FP8, tag="wvl")
        wvs_sb = persist.tile([128, 4, 2, 256], FP8, tag="wvs")
        wo_sb = persist.tile([128, 2, D], BF16, tag="wo")
        bq_sb = persist.tile([128, 4], F32, tag="bq")
        bv_sb = persist.tile([128, 4, 64], F32, tag="bv")
        dm_sb = persist.tile([128, 128], BF16, tag="dm")
        ut_sb = persist.tile([128, 128], BF16, tag="ut")
        id_sb = persist.tile([128, 128], BF16, tag="id")
        qT = persist.tile([128, 2, S], F32R, tag="qT")
        kT = persist.tile([128, 2, S], F32R, tag="kT")
        vn = persist.tile([128, KT, 4, 65], BF16, tag="vn")
        vnT = persist.tile([128, 2, S], BF16, tag="vnT")

        # ---- startup DMAs, ordered by first use ----
        # weight mt axis is stored in order [0, 2, 1, 3] so the hp0 pair
        # (mt 0 and 2) is one contiguous 256KB DMA.
        xc0 = xpool.tile([128, 4, 2, 512], FP8, tag="xc", name="xc0")
        xl0 = xpool.tile([128, 4, 2, 512], FP8, tag="xl", name="xl0")
        xs0 = xpool.tile([128, 4, 2, 512], FP8, tag="xs", name="xs0")
        # pass 1: wq mt0/mt2 + xc0, split fine for first-byte latency
        nc.scalar.dma_start(wq_sb[:, 0:1], wq8[:, 0:1])
        nc.sync.dma_start(xc0[:, 0:2], xq8[:, 0:2, :, 0:512])
        nc.scalar.dma_start(wq_sb[:, 1:2], wq8[:, 1:2])
        nc.sync.dma_start(xc0[:, 2:4], xq8[:, 2:4, :, 0:512])
        nc.scalar.dma_start(wv_sb[:], wv8)
        # pass 2: wql mt0/mt2 + xs0
        nc.scalar.dma_start(wql_sb[:, 0:2], wq8l[:, 0:2])
        nc.sync.dma_start(xs0[:], xs8[:, :, :, 0:512])
        nc.scalar.dma_start(wvl_sb[:], wv8l)
        # pass 3: wqs mt0/mt2 + xl0
        nc.scalar.dma_start(wqs_sb[:, 0:2], wq8s[:, 0:2])
        nc.sync.dma_start(xl0[:], xq8l[:, :, :, 0:512])
        nc.sync.dma_start(bq_sb[:], bq)
        nc.sync.dma_start(dm_sb[:], dmsk)
        nc.sync.dma_start(ut_sb[:], utri)
        nc.scalar.dma_start(wvs_sb[:], wv8s)
        nc.sync.dma_start(bv_sb[:], bv)
        # hp1 qk weights (mt 1 and 3 = stored positions 2:4)
        nc.scalar.dma_start(wq_sb[:, 2:4], wq8[:, 2:4])
        nc.scalar.dma_start(wql_sb[:, 2:4], wq8l[:, 2:4])
        nc.scalar.dma_start(wqs_sb[:, 2:4], wq8s[:, 2:4])
        nc.scalar.dma_start(id_sb[:], identb)
        nc.scalar.dma_start(wo_sb[:], woutb)
        # ones column of vn (softmax denominators) via memset, not DMA
        nc.vector.memset(vn[:, :, :, 64:65], 1.0)
        xtiles = {0: (xc0, xl0, xs0)}

        def qkv_dma(qc):
            qs = slice(qc * 512, (qc + 1) * 512)
            xc = xpool.tile([128, 4, 2, 512], FP8, tag="xc", name=f"xc{qc}")
            xl = xpool.tile([128, 4, 2, 512], FP8, tag="xl", name=f"xl{qc}")
            xs = xpool.tile([128, 4, 2, 512], FP8, tag="xs", name=f"xs{qc}")
            nc.sync.dma_start(xc[:], xq8[:, :, :, qs])
            nc.sync.dma_start(xl[:], xq8l[:, :, :, qs])
            nc.sync.dma_start(xs[:], xs8[:, :, :, qs])
            xtiles[qc] = (xc, xl, xs)

        pq_tiles = {}

        def qk_pass(qc, mt, p, tag="q1"):
            """One error-compensation pass (4 DR matmuls) of a q/k tile."""
            xc, xl, xs = xtiles[qc]
            if p == 0:
                pq_tiles[(qc, mt)] = ps.tile([128, 512], F32, tag=tag,
                                             name=f"pq{qc}{mt}")
            pq = pq_tiles[(qc, mt)]
            wsb, xsb = [(wq_sb, xc), (wql_sb, xs), (wqs_sb, xl)][p]
            mtx = MTX[mt]
            for kp in range(4):
                nc.tensor.matmul(
                    pq[:], wsb[:, mtx, kp, :, :], xsb[:, kp, :, :],
                    start=(p == 0 and kp == 0), stop=(p == 2 and kp == 3),
                    perf_mode=DR)

        def qk_bias(qc, mt):
            qs = slice(qc * 512, (qc + 1) * 512)
            pq = pq_tiles.pop((qc, mt))
            dst = (qT if mt < 2 else kT)[:, mt % 2, qs]
            nc.vector.tensor_scalar_add(dst, pq[:], bq_sb[:, mt:mt + 1])

        def qk_tile(qc, mt):
            for p in range(3):
                qk_pass(qc, mt, p)
            qk_bias(qc, mt)

        pv_tiles = {}

        def v_pass(qc, j, p, tag="q1"):
            xc, xl, xs = xtiles[qc]
            if p == 0:
                pv_tiles[(qc, j)] = ps.tile([128, 256], F32, tag=tag,
                                            name=f"pv{qc}{j}")
            pv = pv_tiles[(qc, j)]
            wsb, xsb = [(wv_sb, xc), (wvl_sb, xs), (wvs_sb, xl)][p]
            for kp in range(4):
                nc.tensor.matmul(
                    pv[:], xsb[:, kp, :, j * 128:(j + 1) * 128],
                    wsb[:, kp, :, :],
                    start=(p == 0 and kp == 0), stop=(p == 2 and kp == 3),
                    perf_mode=DR)

        def v_bias(qc, j):
            st = 4 * qc + j
            pv = pv_tiles.pop((qc, j))
            nc.vector.tensor_add(
                vn[:, st, :, 0:64],
                pv[:].rearrange("p (h d) -> p h d", h=4),
                bv_sb[:])

        def v_tile(qc, j):
            st = 4 * qc + j
            xc, xl, xs = xtiles[qc]
            pv = ps.tile([128, 256], F32, tag="q1", name=f"pv{qc}{j}")
            passes = [(wv_sb, xc), (wvl_sb, xs), (wvs_sb, xl)]
            i = 0
            for wsb, xsb in passes:
                for kp in range(4):
                    nc.tensor.matmul(
                        pv[:], xsb[:, kp, :, j * 128:(j + 1) * 128],
                        wsb[:, kp, :, :],
                        start=(i == 0), stop=(i == 11), perf_mode=DR)
                    i += 1
            nc.vector.tensor_add(
                vn[:, st, :, 0:64],
                pv[:].rearrange("p (h d) -> p h d", h=4),
                bv_sb[:])

        vst_tiles = {}

        def tr_piece(qc, qt, dhs=(0, 1), copy_eng=None, via_dma=False):
            vst = vst_tiles[qc]
            for dh in dhs:
                dst = vnT[:, dh, qc * 512 + qt * 128:qc * 512 + (qt + 1) * 128]
                if via_dma:
                    # SBUF->SBUF crossbar transpose on the DMA path: no PE
                    # or DVE time, fine for latency-insensitive pieces
                    nc.sync.dma_start_transpose(dst, vst[:, qt, 2 * dh:2 * dh + 2, :])
                    continue
                ptr = ps.tile([128, 128], BF16, tag="q1", name=f"tr{qc}{qt}{dh}")
                nc.tensor.matmul(ptr[:], vst[:, qt, 2 * dh:2 * dh + 2, :],
                                 id_sb[:], is_transpose=True)
                eng = copy_eng or nc.vector
                if eng is nc.scalar:
                    eng.copy(dst, ptr[:])
                else:
                    eng.tensor_copy(dst, ptr[:])

        def op_pair(qc, mp, tags=("q1", "q1"), engs=None, split_dma=False):
            """Out-proj for heads-pair mp (m = 2mp, 2mp+1): 4 matmuls, two
            psum->sbuf copies, ONE fused output DMA (HWDGE is a single
            global device at ~630ns per DMA, so fewer DMAs win)."""
            qs = slice(qc * 512, (qc + 1) * 512)
            ou = opool.tile([128, 2, 512], BF16, tag="ou", name=f"ou{qc}{mp}")
            for j, m in enumerate((2 * mp, 2 * mp + 1)):
                pu = ps.tile([128, 512], F32, tag=tags[j], name=f"pu{qc}{m}")
                for t in range(2):
                    nc.tensor.matmul(pu[:], wo_sb[:, t, m * 128:(m + 1) * 128],
                                     vnT[:, t, qs], start=(t == 0), stop=(t == 1))
                eng = engs[j] if engs else nc.vector
                if eng is nc.scalar:
                    eng.copy(ou[:, j], pu[:])
                else:
                    eng.tensor_copy(ou[:, j], pu[:])
                if split_dma:
                    (nc.sync if j == 0 else nc.scalar).dma_start(
                        outT[:, m, qs], ou[:, j])
            if not split_dma:
                (nc.sync if mp % 2 == 0 else nc.scalar).dma_start(
                    outT[:, 2 * mp:2 * mp + 2, qs], ou[:])

        def op_tail(qc, mp):
            """Tail out-proj pair: pu psum uses the q1 and (now free) s tags;
            copies round-robin DVE/ACT/GPSIMD to pipeline behind PE."""
            engs = [(nc.vector, nc.scalar), (nc.vector, nc.scalar),
                    (nc.scalar, nc.vector), (nc.vector, nc.scalar)][mp]
            op_pair(qc, mp, tags=("q1", "s"), engs=engs, split_dma=(mp == 3))

        queue = _Q()

        def push_qkv_late(c):
            for j in (2, 3):
                queue.push(lambda c=c, j=j: v_tile(c, j), f"v{c}{j}")
            for mt in (1, 3):
                queue.push(lambda c=c, mt=mt: qk_tile(c, mt), f"qk{c}{mt}")

        def push_qkv_early(c):
            queue.push(lambda c=c: qkv_dma(c), f"dma{c}")
            for mt in (0, 2):
                queue.push(lambda c=c, mt=mt: qk_tile(c, mt), f"qk{c}{mt}")
            for j in (0, 1):
                queue.push(lambda c=c, j=j: v_tile(c, j), f"v{c}{j}")

        def push_post(c):
            for qt in range(4):
                queue.push(lambda c=c, qt=qt: tr_piece(c, qt), f"tr{c}{qt}")
            for mp in range(4):
                queue.push(lambda c=c, mp=mp: op_pair(c, mp), f"op{c}{mp}")

        def sc_of(qc, hp, ki):
            """Scores + exp for one k-tile of group (qc, hp): causal-mask
            matmul (diag tiles), f32r score matmuls, ACT exp -> e tile."""
            j = ki - 4 * qc
            o_exp = max(0, 128 * j)
            o_sc = min(o_exp, 256)  # f32r moving dim must be >= 256
            sp = ps.tile([128, 2, 512], F32, tag="s", name=f"sp{qc}{hp}{ki}")
            for i in range(2):
                vp = 64 * i
                if j >= 0:
                    # causal mask: psum[k, q] -= 1e9 * [k > q] on the
                    # diagonal block, via diag(-1e9) @ strict-upper-tri
                    nc.tensor.matmul(
                        sp[:, i, o_exp:o_exp + 128], dm_sb[:], ut_sb[:],
                        start=True, stop=False, skip_group_check=True)
                nc.tensor.matmul(
                    sp[:, i, o_sc:512],
                    kT[vp:vp + 64, hp, ki * 128:(ki + 1) * 128],
                    qT[vp:vp + 64, hp, qc * 512 + o_sc:(qc + 1) * 512],
                    start=(j < 0), stop=True, tile_position=(vp, 0),
                    skip_group_check=True)
            e = epool.tile([128, 2, 512], BF16, tag="e", name=f"e{qc}{hp}{ki}")
            nc.scalar.activation(e[:, :, o_exp:512], sp[:, :, o_exp:512],
                                 EXP, scale=0.125)
            return e

        def attn_group(qc, hp, inline=None, per_step=0.0, tail=False,
                       need=None, pre=None, next_hook=None):
            """Attention for group (qc, hp) with score-ahead pipelining.

            inline: optional dict ki -> [fn] of pieces emitted right before
            AV(ki) (used for chunk 0's v tiles).  need: dict ki -> queue
            label that must be emitted before AV(ki) (vn dependencies).
            pre: e tiles {0,1} pre-emitted by the previous group's tail.
            next_hook: called at ki == n_ki-2 to pre-emit the NEXT group's
            first scores so ACT has no bubble at the group boundary;
            its return value is returned.  tail=True pipelines the last
            chunk's normalize/transpose/out-proj per qt.
            """
            vst = vst_tiles[qc]
            n_ki = 4 * qc + 4

            def sc(ki):
                return sc_of(qc, hp, ki)

            def av(ki, e):
                j = ki - 4 * qc
                for i in range(2):
                    for qt in range(max(0, j), 4):
                        nc.tensor.matmul(
                            po[:, i, qt * 65:qt * 65 + 65],
                            e[:, i, qt * 128:(qt + 1) * 128],
                            vn[:, ki, 2 * hp + i, :],
                            start=(ki == 0 and qt == 0),
                            stop=(ki == 4 * qc + qt),
                            skip_group_check=True)

            def norm_qt(qt):
                for i in range(2):
                    dn = po[:, i, 0:260].rearrange("p (qt c) -> p qt c", c=65)
                    with nc.allow_low_precision(reason="softmax recip"):
                        nc.vector.reciprocal(rc[:, i, qt:qt + 1],
                                             dn[:, qt, 64:65])
                    nc.vector.tensor_scalar_mul(
                        vst[:, qt, 2 * hp + i, :],
                        po[:, i, qt * 65:qt * 65 + 64],
                        rc[:, i, qt:qt + 1])

            def norm_all():
                for i in range(2):
                    dn = po[:, i, 0:260].rearrange("p (qt c) -> p qt c", c=65)
                    with nc.allow_low_precision(reason="softmax recip"):
                        nc.vector.reciprocal(rc[:, i, :], dn[:, 0:4, 64:65])
                    for qt in range(4):
                        nc.vector.tensor_scalar_mul(
                            vst[:, qt, 2 * hp + i, :],
                            po[:, i, qt * 65:qt * 65 + 64],
                            rc[:, i, qt:qt + 1])

            po = ps.tile([128, 2, 512], F32, tag="po", name=f"po{qc}{hp}", bufs=1)
            rc = spool.tile([128, 2, 4], F32, tag="rc", name=f"rc{qc}{hp}")
            es = dict(pre) if pre else {}
            if 0 not in es:
                es[0] = sc(0)
            if n_ki > 1 and 1 not in es:
                es[1] = sc(1)
            pre_next = None
            for ki in range(n_ki):
                if inline:
                    for fn in inline.get(ki, ()):
                        fn()
                if need and ki in need:
                    queue.drain_to(need[ki])
                av(ki, es.pop(ki))
                if ki + 2 < n_ki:
                    es[ki + 2] = sc(ki + 2)
                if next_hook and ki == n_ki - _HOOKLAG:
                    pre_next = next_hook()
                if tail and ki >= n_ki - 4:
                    qt = ki - (n_ki - 4)
                    norm_qt(qt)
                    tr_piece(qc, qt, dhs=(0,), copy_eng=nc.scalar)
                    tr_piece(qc, qt, dhs=(1,), copy_eng=nc.vector)
                else:
                    queue.drain_frac(per_step)
            if tail:
                for mp in range(4):
                    op_tail(qc, mp)
            else:
                norm_all()
            return pre_next

        # ---- chunk 0: hp0 qk + v0/v1 pass-major so PE has work while
        # pass-2/3 bytes stream in and first scores start ASAP ----
        for p in range(3):
            qk_pass(0, 0, p, tag="s")
            qk_pass(0, 2, p, tag="s")
            v_pass(0, 0, p)
            v_pass(0, 1, p)
        qk_bias(0, 0)
        qk_bias(0, 2)
        v_bias(0, 0)
        v_bias(0, 1)

        vst_tiles[0] = spool.tile([128, 4, 4, 64], BF16, tag="vst", name="vs0")
        # chunk 0's v tiles run inline between AVs; only qk hp1 is queued
        for mt in (1, 3):
            queue.push(lambda mt=mt: qk_tile(0, mt), f"qk0{mt}")
        push_qkv_early(1)
        inline0 = {ki: [lambda ki=ki: v_tile(0, ki)] for ki in (2, 3)}

        def hook_for(qc2, hp2, drains):
            """Pre-emit drains + the first two scores of group (qc2, hp2)."""
            def h():
                for d in drains:
                    queue.drain_to(d)
                es = {0: sc_of(qc2, hp2, 0)}
                if 4 * qc2 + 4 > 1:
                    es[1] = sc_of(qc2, hp2, 1)
                return es
            return h

        pre = attn_group(0, 0, inline=inline0,
                         per_step=_PACE[0] * queue.remaining() / 4,
                         next_hook=hook_for(0, 1, ["qk03"]))

        for qc in range(SC):
            n_ki = 4 * qc + 4
            if qc > 0:
                vst_tiles[qc] = spool.tile([128, 4, 4, 64], BF16, tag="vst",
                                           name=f"vs{qc}")
                push_qkv_late(qc)
                if qc + 1 < SC:
                    push_qkv_early(qc + 1)
                push_post(qc - 1)
                # scores need this chunk's qT/kT hp0; AV(ki) needs vn[ki]
                queue.drain_to(f"qk{qc}2")
                need = {max(0, 4 * qc + j - 2): f"v{qc}{j}" for j in range(4)}
                f0 = _PACE[3] if qc == SC - 1 else _PACE[1]
                pre = attn_group(qc, 0, need=need, pre=pre,
                                 per_step=f0 * queue.remaining() / n_ki,
                                 next_hook=hook_for(qc, 1, [f"qk{qc}3"]))
            # hp1 needs this chunk's mt=1,3 projections emitted first
            queue.drain_to(f"qk{qc}3")
            if qc < SC - 1:
                nh = hook_for(qc + 1, 0, [f"qk{qc + 1}2"])
                pre = attn_group(qc, 1, pre=pre, next_hook=nh,
                                 per_step=_PACE[2] * queue.remaining() / n_ki)
            else:
                attn_group(qc, 1, tail=True, pre=pre,
                           per_step=queue.remaining() / (n_ki - 4))

        queue.flush()
        if dbg:
            nc.sync.dma_start(d_vst, vst_tiles[0][:])
            nc.sync.dma_start(d_qT, qT[:])
            nc.sync.dma_start(d_kT, kT[:])
            nc.sync.dma_start(d_vn, vn[:])
            nc.sync.dma_start(d_vnT, vnT[:])

    if fix_waits:
        _fix_sync_waits(nc)
    return nc


def _get_nc():
    if "nc" not in _CACHE:
        _CACHE["nc"] = _build()
    return _CACHE["nc"]


def _dr_layout(xb):
    """[S, 1024] -> [128, 4, 2, S]: p=partition, kp=k-tile-pair, sl=slot."""
    return np.ascontiguousarray(
        xb.T.reshape(4, 2, 128, xb.shape[0]).transpose(2, 0, 1, 3))


def kernel(x, W_qkv, b_qkv, W_out, b_out):
    x = np.asarray(x, np.float32)
    W_qkv = np.asarray(W_qkv, np.float32)
    b_qkv = np.asarray(b_qkv, np.float32)
    W_out = np.asarray(W_out, np.float32)
    b_out = np.asarray(b_out, np.float32)

    nc = _get_nc()

    kk = np.arange(128)[:, None]
    qq = np.arange(128)[None, :]
    dmask = (-1e9 * np.eye(128, dtype=np.float32)).astype(BF)
    utri = (kk > qq).astype(BF)      # [r, q] = 1 where r > q
    identb = np.eye(128, dtype=np.float32).astype(BF)

    in_maps = []
    for c in range(N_CORES):
        b, g = divmod(c, 4)
        heads = [4 * g + i for i in range(HL)]

        xb = x[b]                                        # [S, 1024]
        xr = _dr_layout(xb)
        x8 = xr.astype(E4)
        x8l = ((xr - x8.astype(np.float32)) * 8.0).astype(E4)
        xs8_a = (xr * 0.125).astype(E4)

        # qk weight m-tiles: mt0=q-hp0, mt1=q-hp1, mt2=k-hp0, mt3=k-hp1
        # out-col within tile = 64*i + dd  (i head-in-pair, dd hd index)
        wq = np.zeros((1024, 4, 128), np.float32)
        bqv = np.zeros((128, 4), np.float32)
        for mt in range(4):
            t, hp = divmod(mt, 2)       # t: 0=q, 1=k
            for i in range(2):
                h = heads[2 * hp + i]
                cols = h * 192 + 64 * t + np.arange(64)
                wq[:, mt, 64 * i:64 * i + 64] = W_qkv[:, cols]
                bqv[64 * i:64 * i + 64, mt] = b_qkv[cols]
        # mt axis stored as [0,2,1,3]; [1024, mt, 128] -> [128(p), mt, kp, sl, 128]
        wq = wq[:, [0, 2, 1, 3], :]
        wq = wq.reshape(4, 2, 128, 4, 128).transpose(2, 3, 0, 1, 4)
        wq8 = wq.astype(E4)
        wq8l = ((wq - wq8.astype(np.float32)) * 8.0).astype(E4)
        wq8s = (wq * 0.125).astype(E4)

        # v weights: col = 64*h + dd
        wv = np.zeros((1024, 256), np.float32)
        bvv = np.zeros((4, 64), np.float32)
        for hh in range(4):
            cols = heads[hh] * 192 + 128 + np.arange(64)
            wv[:, 64 * hh:64 * hh + 64] = W_qkv[:, cols]
            bvv[hh] = b_qkv[cols]
        wv = wv.reshape(4, 2, 128, 256).transpose(2, 0, 1, 3)
        wv8 = wv.astype(E4)
        wv8l = ((wv - wv8.astype(np.float32)) * 8.0).astype(E4)
        wv8s = (wv * 0.125).astype(E4)
        bv2 = np.broadcast_to(bvv[None], (128, 4, 64))

        wo = W_out[g * 256:(g + 1) * 256, :]             # [256, 1024]
        wob = wo.reshape(2, 128, D).transpose(1, 0, 2).astype(BF)

        in_maps.append({
            "xq8": x8,
            "xq8l": x8l,
            "xs8": xs8_a,
            "wq8": np.ascontiguousarray(wq8),
            "wq8l": np.ascontiguousarray(wq8l),
            "wq8s": np.ascontiguousarray(wq8s),
            "wv8": np.ascontiguousarray(wv8),
            "wv8l": np.ascontiguousarray(wv8l),
            "wv8s": np.ascontiguousarray(wv8s),
            "woutb": np.ascontiguousarray(wob),
            "bq": np.ascontiguousarray(bqv),
            "bv": np.ascontiguousarray(bv2),
            "dmsk": np.ascontiguousarray(dmask),
            "utri": np.ascontiguousarray(utri),
            "identb": identb,
        })

    _CACHE["in_maps"] = in_maps
    res = bass_utils.run_bass_kernel_spmd(nc, in_maps, core_ids=list(range(N_CORES)))

    out = np.zeros((B, S, D), np.float32)
    for c in range(N_CORES):
        b = c // 4
        oT = np.asarray(res.results[c]["outT"]).astype(np.float32)
        out[b] += oT.transpose(1, 0, 2).reshape(D, S).T
    out += b_out
    return out


# revision 5
# speedup vs baseline: 1.0739x; 1.0148x over previous
"""Multi-head causal attention (B=2, S=2048, D=1024, H=16) on 8 TRN2 NeuronCores.

Sharding: core c handles batch b = c // 4 and local head group g = c % 4
(global heads 4g..4g+3).  Each core computes its heads' QKV projections,
causal attention, and a partial output projection; host sums the 4 partials
per batch and adds b_out.

v3 design, 120.3us TimelineSim (v2 baseline 129.2us):
  - Score-ahead pipelining: scores for ki+2 are emitted before AV(ki) in PE
    program order, so ACT (exp) runs back-to-back instead of ping-ponging
    with PE.  exp is the per-ki long pole (1024 els x 0.83ns vs PE 644ns).
  - Group-boundary pre-emit: the next (qc, hp) group's first two
    score+exp tiles are emitted inside the current group's tail (next_hook)
    so ACT has no bubble across hp/chunk transitions.
  - Causal mask folded into the scores psum accumulation as a PE matmul:
    diag(-1e9) @ strict-upper-tri accumulated before the f32r score matmul.
    exp(-1.25e8) = 0, so the post-exp DVE mask multiply is gone.
  - Psum pending-zero folded into the first AV matmul of each bank
    (start=True zeroes the bank) -- the zro dummy matmuls are gone.
  - Startup: weight mt axis stored [0,2,1,3] so hp0 slices are single DMAs,
    DMAs ordered by first use (few and large: HWDGE is a single global
    ~630ns/DMA device); chunk-0 hp0 QK + v0/v1 projections run pass-major
    so PE has work while pass-2/3 bytes stream in.
  - Output DMAs fused per head-pair (one [128,2,512] DMA per two m tiles).
  - Last chunk tail: per-qt normalize/transpose as each AV accumulation
    stops (ki = 12+qt); out-proj pairs use the freed s-tag psum slots for
    ring depth 4 with copies round-robin DVE/ACT.
  - Explicit drain-ordering (labels) replaces pacing-only correctness.
  - NOTE: GPSIMD cannot access PSUM on TRN2 (BIR verifier) -- all
    psum->sbuf moves must be on DVE or ACT.
"""

from contextlib import ExitStack

import numpy as np
import ml_dtypes

import concourse.bass as bass
import concourse.mybir as mybir
import concourse.tile as tile
from concourse import bass_utils

F32 = mybir.dt.float32
F32R = mybir.dt.float32r
BF16 = mybir.dt.bfloat16
FP8 = mybir.dt.float8e4
EXP = mybir.ActivationFunctionType.Exp
COPY = mybir.ActivationFunctionType.Copy
DR = mybir.MatmulPerfMode.DoubleRow

E4 = ml_dtypes.float8_e4m3
BF = ml_dtypes.bfloat16

B, S, D, H = 2, 2048, 1024, 16
HD = D // H          # 64
HL = 4               # heads per core
N_CORES = 8
SC = S // 512        # 4 q-chunks of 512
KT = S // 128        # 16 k-tiles of 128
MTX = {0: 0, 2: 1, 1: 2, 3: 3}  # mt -> stored position (hp0 pair first)

_CACHE = {}
_PACE = [0.4, 0.5, 0.6, 0.4]
_HOOKLAG = 2


def _round_f32r(x: np.ndarray) -> np.ndarray:
    """Round f32 to fp32r (11-bit mantissa, RNE) on host."""
    u = np.ascontiguousarray(x, dtype=np.float32).view(np.uint32)
    frac = u & np.uint32(0x00000FFF)
    base = u & np.uint32(0xFFFFF000)
    bit = np.uint32(0x00000800)
    lsb = np.uint32(0x00001000)
    roundup = (frac > bit) | ((frac == bit) & ((u & lsb) != 0))
    return np.where(roundup, base + lsb, base).view(np.float32)


_NO_HOIST = {
    "AllEngineBarrier",
    "EventSemaphore",
    "UnconditionalBranch",
    "CompareAndBranch",
    "BranchHint",
    "IndirectBranch",
    "Halt",
    "Call",
    "OverlayCall",
    "NoOp",
}


def _fix_sync_waits(nc):
    """walrus codegen holds only one sync-wait per engine instruction; hoist
    excess waits onto same-engine NoOps inserted right before."""
    for fn in nc.m.functions:
        for blk in fn.blocks:
            insts = blk.instructions
            out = []
            changed = False
            for inst in insts:
                si = inst.sync_info
                if si is not None and inst.opcode not in _NO_HOIST:
                    waits = list(si.on_wait)
                    if len(waits) > 1:
                        for j, w in enumerate(waits[:-1]):
                            nop = mybir.InstNoOp(name=f"{inst.name}-wfix{j}")
                            nop.engine = inst.engine
                            nop.sync_info = mybir.SyncInfo(on_wait=[w], on_update=[])
                            out.append(nop)
                        inst.sync_info = mybir.SyncInfo(
                            on_wait=[waits[-1]], on_update=list(si.on_update)
                        )
                        changed = True
                out.append(inst)
            if changed:
                blk.instructions = out


class _Q:
    """Emission-time work queue with credit pacing + forced ordering."""

    def __init__(self):
        self.items = []      # (fn, label)
        self.qi = 0
        self.credit = 0.0

    def push(self, fn, label=None):
        self.items.append((fn, label))

    def remaining(self):
        return len(self.items) - self.qi

    def _emit_one(self):
        fn, _ = self.items[self.qi]
        self.qi += 1
        fn()

    def drain_frac(self, frac):
        self.credit += frac
        while self.qi < len(self.items) and self.qi < self.credit:
            self._emit_one()

    def drain_to(self, label):
        """Emit everything up to and including the piece tagged `label`."""
        done = any(lb == label for _, lb in self.items[: self.qi])
        if done:
            return
        while self.qi < len(self.items):
            lb = self.items[self.qi][1]
            self._emit_one()
            self.credit = max(self.credit, self.qi)
            if lb == label:
                return
        raise KeyError(f"label {label} not found in queue")

    def flush(self):
        while self.qi < len(self.items):
            self._emit_one()
        self.credit = self.qi


def _build(fix_waits=True, dbg=False):
    nc = bass.Bass("TRN2", target_bir_lowering=False, debug=False,
                   num_devices=N_CORES)
    if dbg:
        d_qT = nc.dram_tensor("d_qT", [128, 2, S], F32R, kind="ExternalOutput").ap()
        d_kT = nc.dram_tensor("d_kT", [128, 2, S], F32R, kind="ExternalOutput").ap()
        d_vn = nc.dram_tensor("d_vn", [128, KT, 4, 65], BF16,
                              kind="ExternalOutput").ap()
        d_vst = nc.dram_tensor("d_vst", [128, 4, 4, 64], BF16,
                               kind="ExternalOutput").ap()
        d_vnT = nc.dram_tensor("d_vnT", [128, 2, S], BF16,
                               kind="ExternalOutput").ap()

    # x in fp8 hi / lo*8 / /8 copies, [128, kp, sl, S]
    xq8 = nc.dram_tensor("xq8", [128, 4, 2, S], FP8, kind="ExternalInput").ap()
    xq8l = nc.dram_tensor("xq8l", [128, 4, 2, S], FP8, kind="ExternalInput").ap()
    xs8 = nc.dram_tensor("xs8", [128, 4, 2, S], FP8, kind="ExternalInput").ap()
    # qk weights mt-major: [128, mt, kp, sl, 128]
    wq8 = nc.dram_tensor("wq8", [128, 4, 4, 2, 128], FP8, kind="ExternalInput").ap()
    wq8l = nc.dram_tensor("wq8l", [128, 4, 4, 2, 128], FP8, kind="ExternalInput").ap()
    wq8s = nc.dram_tensor("wq8s", [128, 4, 4, 2, 128], FP8, kind="ExternalInput").ap()
    wv8 = nc.dram_tensor("wv8", [128, 4, 2, 256], FP8, kind="ExternalInput").ap()
    wv8l = nc.dram_tensor("wv8l", [128, 4, 2, 256], FP8, kind="ExternalInput").ap()
    wv8s = nc.dram_tensor("wv8s", [128, 4, 2, 256], FP8, kind="ExternalInput").ap()
    woutb = nc.dram_tensor("woutb", [128, 2, D], BF16, kind="ExternalInput").ap()
    bq = nc.dram_tensor("bq", [128, 4], F32, kind="ExternalInput").ap()
    bv = nc.dram_tensor("bv", [128, 4, 64], F32, kind="ExternalInput").ap()
    dmsk = nc.dram_tensor("dmsk", [128, 128], BF16, kind="ExternalInput").ap()
    utri = nc.dram_tensor("utri", [128, 128], BF16, kind="ExternalInput").ap()
    identb = nc.dram_tensor("identb", [128, 128], BF16, kind="ExternalInput").ap()
    outT = nc.dram_tensor("outT", [128, 8, S], BF16, kind="ExternalOutput").ap()

    with tile.TileContext(nc) as tc, ExitStack() as ctx:
        persist = ctx.enter_context(tc.tile_pool(name="persist", bufs=1))
        xpool = ctx.enter_context(tc.tile_pool(name="xp", bufs=3))
        epool = ctx.enter_context(tc.tile_pool(name="ep", bufs=8))
        spool = ctx.enter_context(tc.tile_pool(name="stp", bufs=3))
        opool = ctx.enter_context(tc.tile_pool(name="op", bufs=6))
        # psum (8 banks): s 2x2-bank (sp / startup pq), po 1x2-bank,
        # q1 2x1-bank (pv/pq/pu/tr churn)
        ps = ctx.enter_context(tc.tile_pool(name="ps", bufs=2, space="PSUM"))

        wq_sb = persist.tile([128, 4, 4, 2, 128], FP8, tag="wq")
        wql_sb = persist.tile([128, 4, 4, 2, 128], FP8, tag="wql")
        wqs_sb = persist.tile([128, 4, 4, 2, 128], FP8, tag="wqs")
        wv_sb = persist.tile([128, 4, 2, 256], FP8, tag="wv")
        wvl_sb = persist.tile([128, 4, 2, 256], FP8, tag="wvl")
        wvs_sb = persist.tile([128, 4, 2, 256], FP8, tag="wvs")
        wo_sb = persist.tile([128, 2, D], BF16, tag="wo")
        bq_sb = persist.tile([128, 4], F32, tag="bq")
        bv_sb = persist.tile([128, 4, 64], F32, tag="bv")
        dm_sb = persist.tile([128, 128], BF16, tag="dm")
        ut_sb = persist.tile([128, 128], BF16, tag="ut")
        id_sb = persist.tile([128, 128], BF16, tag="id")
        qT = persist.tile([128, 2, S], F32R, tag="qT")
        kT = persist.tile([128, 2, S], F32R, tag="kT")
        vn = persist.tile([128, KT, 4, 65], BF16, tag="vn")
        vnT = persist.tile([128, 2, S], BF16, tag="vnT")

        # ---- startup DMAs, ordered by first use ----
        # weight mt axis is stored in order [0, 2, 1, 3] so the hp0 pair
        # (mt 0 and 2) is one contiguous 256KB DMA.
        xc0 = xpool.tile([128, 4, 2, 512], FP8, tag="xc", name="xc0")
        xl0 = xpool.tile([128, 4, 2, 512], FP8, tag="xl", name="xl0")
        xs0 = xpool.tile([128, 4, 2, 512], FP8, tag="xs", name="xs0")
        # pass 1: wq mt0/mt2 + xc0, split fine for first-byte latency
        nc.scalar.dma_start(wq_sb[:, 0:1], wq8[:, 0:1])
        nc.sync.dma_start(xc0[:, 0:2], xq8[:, 0:2, :, 0:512])
        nc.scalar.dma_start(wq_sb[:, 1:2], wq8[:, 1:2])
        nc.sync.dma_start(xc0[:, 2:4], xq8[:, 2:4, :, 0:512])
        nc.scalar.dma_start(wv_sb[:], wv8)
        # pass 2: wql mt0/mt2 + xs0
        nc.scalar.dma_start(wql_sb[:, 0:2], wq8l[:, 0:2])
        nc.sync.dma_start(xs0[:], xs8[:, :, :, 0:512])
        nc.scalar.dma_start(wvl_sb[:], wv8l)
        # pass 3: wqs mt0/mt2 + xl0
        nc.scalar.dma_start(wqs_sb[:, 0:2], wq8s[:, 0:2])
        nc.sync.dma_start(xl0[:], xq8l[:, :, :, 0:512])
        nc.sync.dma_start(bq_sb[:], bq)
        nc.sync.dma_start(dm_sb[:], dmsk)
        nc.sync.dma_start(ut_sb[:], utri)
        nc.scalar.dma_start(wvs_sb[:], wv8s)
        nc.sync.dma_start(bv_sb[:], bv)
        # hp1 qk weights (mt 1 and 3 = stored positions 2:4)
        nc.scalar.dma_start(wq_sb[:, 2:4], wq8[:, 2:4])
        nc.scalar.dma_start(wql_sb[:, 2:4], wq8l[:, 2:4])
        nc.scalar.dma_start(wqs_sb[:, 2:4], wq8s[:, 2:4])
        nc.scalar.dma_start(id_sb[:], identb)
        nc.scalar.dma_start(wo_sb[:], woutb)
        # ones column of vn (softmax denominators) via memset, not DMA
        nc.vector.memset(vn[:, :, :, 64:65], 1.0)
        xtiles = {0: (xc0, xl0, xs0)}

        def qkv_dma(qc):
            qs = slice(qc * 512, (qc + 1) * 512)
            xc = xpool.tile([128, 4, 2, 512], FP8, tag="xc", name=f"xc{qc}")
            xl = xpool.tile([128, 4, 2, 512], FP8, tag="xl", name=f"xl{qc}")
            xs = xpool.tile([128, 4, 2, 512], FP8, tag="xs", name=f"xs{qc}")
            nc.sync.dma_start(xc[:], xq8[:, :, :, qs])
            nc.sync.dma_start(xl[:], xq8l[:, :, :, qs])
            nc.sync.dma_start(xs[:], xs8[:, :, :, qs])
            xtiles[qc] = (xc, xl, xs)

        pq_tiles = {}

        def qk_pass(qc, mt, p, tag="q1"):
            """One error-compensation pass (4 DR matmuls) of a q/k tile."""
            xc, xl, xs = xtiles[qc]
            if p == 0:
                pq_tiles[(qc, mt)] = ps.tile([128, 512], F32, tag=tag,
                                             name=f"pq{qc}{mt}")
            pq = pq_tiles[(qc, mt)]
            wsb, xsb = [(wq_sb, xc), (wql_sb, xs), (wqs_sb, xl)][p]
            mtx = MTX[mt]
            for kp in range(4):
                nc.tensor.matmul(
                    pq[:], wsb[:, mtx, kp, :, :], xsb[:, kp, :, :],
                    start=(p == 0 and kp == 0), stop=(p == 2 and kp == 3),
                    perf_mode=DR)

        def qk_bias(qc, mt):
            qs = slice(qc * 512, (qc + 1) * 512)
            pq = pq_tiles.pop((qc, mt))
            dst = (qT if mt < 2 else kT)[:, mt % 2, qs]
            nc.vector.tensor_scalar_add(dst, pq[:], bq_sb[:, mt:mt + 1])

        def qk_tile(qc, mt):
            for p in range(3):
                qk_pass(qc, mt, p)
            qk_bias(qc, mt)

        pv_tiles = {}

        def v_pass(qc, j, p, tag="q1"):
            xc, xl, xs = xtiles[qc]
            if p == 0:
                pv_tiles[(qc, j)] = ps.tile([128, 256], F32, tag=tag,
                                            name=f"pv{qc}{j}")
            pv = pv_tiles[(qc, j)]
            wsb, xsb = [(wv_sb, xc), (wvl_sb, xs), (wvs_sb, xl)][p]
            for kp in range(4):
                nc.tensor.matmul(
                    pv[:], xsb[:, kp, :, j * 128:(j + 1) * 128],
                    wsb[:, kp, :, :],
                    start=(p == 0 and kp == 0), stop=(p == 2 and kp == 3),
                    perf_mode=DR)

        def v_bias(qc, j):
            st = 4 * qc + j
            pv = pv_tiles.pop((qc, j))
            nc.vector.tensor_add(
                vn[:, st, :, 0:64],
                pv[:].rearrange("p (h d) -> p h d", h=4),
                bv_sb[:])

        def v_tile(qc, j):
            st = 4 * qc + j
            xc, xl, xs = xtiles[qc]
            pv = ps.tile([128, 256], F32, tag="q1", name=f"pv{qc}{j}")
            passes = [(wv_sb, xc), (wvl_sb, xs), (wvs_sb, xl)]
            i = 0
            for wsb, xsb in passes:
                for kp in range(4):
                    nc.tensor.matmul(
                        pv[:], xsb[:, kp, :, j * 128:(j + 1) * 128],
                        wsb[:, kp, :, :],
                        start=(i == 0), stop=(i == 11), perf_mode=DR)
                    i += 1
            nc.vector.tensor_add(
                vn[:, st, :, 0:64],
                pv[:].rearrange("p (h d) -> p h d", h=4),
                bv_sb[:])

        vst_tiles = {}

        def tr_piece(qc, qt, dhs=(0, 1), copy_eng=None, via_dma=False):
            vst = vst_tiles[qc]
            for dh in dhs:
                dst = vnT[:, dh, qc * 512 + qt * 128:qc * 512 + (qt + 1) * 128]
                if via_dma:
                    # SBUF->SBUF crossbar transpose on the DMA path: no PE
                    # or DVE time, fine for latency-insensitive pieces
                    nc.sync.dma_start_transpose(dst, vst[:, qt, 2 * dh:2 * dh + 2, :])
                    continue
                ptr = ps.tile([128, 128], BF16, tag="q1", name=f"tr{qc}{qt}{dh}")
                nc.tensor.matmul(ptr[:], vst[:, qt, 2 * dh:2 * dh + 2, :],
                                 id_sb[:], is_transpose=True)
                eng = copy_eng or nc.vector
                if eng is nc.scalar:
                    eng.copy(dst, ptr[:])
                else:
                    eng.tensor_copy(dst, ptr[:])

        def op_pair(qc, mp, tags=("q1", "q1"), engs=None, split_dma=False):
            """Out-proj for heads-pair mp (m = 2mp, 2mp+1): 4 matmuls, two
            psum->sbuf copies, ONE fused output DMA (HWDGE is a single
            global device at ~630ns per DMA, so fewer DMAs win)."""
            qs = slice(qc * 512, (qc + 1) * 512)
            ou = opool.tile([128, 2, 512], BF16, tag="ou", name=f"ou{qc}{mp}")
            for j, m in enumerate((2 * mp, 2 * mp + 1)):
                pu = ps.tile([128, 512], F32, tag=tags[j], name=f"pu{qc}{m}")
                for t in range(2):
                    nc.tensor.matmul(pu[:], wo_sb[:, t, m * 128:(m + 1) * 128],
                                     vnT[:, t, qs], start=(t == 0), stop=(t == 1))
                eng = engs[j] if engs else nc.vector
                if eng is nc.scalar:
                    eng.copy(ou[:, j], pu[:])
                else:
                    eng.tensor_copy(ou[:, j], pu[:])
                if split_dma:
                    (nc.sync if j == 0 else nc.scalar).dma_start(
                        outT[:, m, qs], ou[:, j])
            if not split_dma:
                (nc.sync if mp % 2 == 0 else nc.scalar).dma_start(
                    outT[:, 2 * mp:2 * mp + 2, qs], ou[:])

        def op_tail(qc, mp):
            """Tail out-proj pair: pu psum uses the q1 and (now free) s tags;
            copies round-robin DVE/ACT/GPSIMD to pipeline behind PE."""
            engs = [(nc.vector, nc.scalar), (nc.vector, nc.scalar),
                    (nc.scalar, nc.vector), (nc.vector, nc.scalar)][mp]
            op_pair(qc, mp, tags=("q1", "s"), engs=engs, split_dma=(mp == 3))

        queue = _Q()

        def push_qkv_late(c):
            for j in (2, 3):
                queue.push(lambda c=c, j=j: v_tile(c, j), f"v{c}{j}")
            for mt in (1, 3):
                queue.push(lambda c=c, mt=mt: qk_tile(c, mt), f"qk{c}{mt}")

        def push_qkv_early(c):
            queue.push(lambda c=c: qkv_dma(c), f"dma{c}")
            for mt in (0, 2):
                queue.push(lambda c=c, mt=mt: qk_tile(c, mt), f"qk{c}{mt}")
            for j in (0, 1):
                queue.push(lambda c=c, j=j: v_tile(c, j), f"v{c}{j}")

        def push_post(c):
            for qt in range(4):
                queue.push(lambda c=c, qt=qt: tr_piece(c, qt), f"tr{c}{qt}")
            for mp in range(4):
                queue.push(lambda c=c, mp=mp: op_pair(c, mp), f"op{c}{mp}")

        def sc_of(qc, hp, ki):
            """Scores + exp for one k-tile of group (qc, hp): causal-mask
            matmul (diag tiles), f32r score matmuls, ACT exp -> e tile."""
            j = ki - 4 * qc
            o_exp = max(0, 128 * j)
            o_sc = min(o_exp, 256)  # f32r moving dim must be >= 256
            sp = ps.tile([128, 2, 512], F32, tag="s", name=f"sp{qc}{hp}{ki}")
            for i in range(2):
                vp = 64 * i
                if j >= 0:
                    # causal mask: psum[k, q] -= 1e9 * [k > q] on the
                    # diagonal block, via diag(-1e9) @ strict-upper-tri
                    nc.tensor.matmul(
                        sp[:, i, o_exp:o_exp + 128], dm_sb[:], ut_sb[:],
                        start=True, stop=False, skip_group_check=True)
                nc.tensor.matmul(
                    sp[:, i, o_sc:512],
                    kT[vp:vp + 64, hp, ki * 128:(ki + 1) * 128],
                    qT[vp:vp + 64, hp, qc * 512 + o_sc:(qc + 1) * 512],
                    start=(j < 0), stop=True, tile_position=(vp, 0),
                    skip_group_check=True)
            e = epool.tile([128, 2, 512], BF16, tag="e", name=f"e{qc}{hp}{ki}")
            nc.scalar.activation(e[:, :, o_exp:512], sp[:, :, o_exp:512],
                                 EXP, scale=0.125)
            return e

        def attn_group(qc, hp, inline=None, per_step=0.0, tail=False,
                       need=None, pre=None, next_hook=None):
            """Attention for group (qc, hp) with score-ahead pipelining.

            inline: optional dict ki -> [fn] of pieces emitted right before
            AV(ki) (used for chunk 0's v tiles).  need: dict ki -> queue
            label that must be emitted before AV(ki) (vn dependencies).
            pre: e tiles {0,1} pre-emitted by the previous group's tail.
            next_hook: called at ki == n_ki-2 to pre-emit the NEXT group's
            first scores so ACT has no bubble at the group boundary;
            its return value is returned.  tail=True pipelines the last
            chunk's normalize/transpose/out-proj per qt.
            """
            vst = vst_tiles[qc]
            n_ki = 4 * qc + 4

            def sc(ki):
                return sc_of(qc, hp, ki)

            def av(ki, e):
                j = ki - 4 * qc
                for i in range(2):
                    for qt in range(max(0, j), 4):
                        nc.tensor.matmul(
                            po[:, i, qt * 65:qt * 65 + 65],
                            e[:, i, qt * 128:(qt + 1) * 128],
                            vn[:, ki, 2 * hp + i, :],
                            start=(ki == 0 and qt == 0),
                            stop=(ki == 4 * qc + qt),
                            skip_group_check=True)

            def norm_qt(qt):
                for i in range(2):
                    dn = po[:, i, 0:260].rearrange("p (qt c) -> p qt c", c=65)
                    with nc.allow_low_precision(reason="softmax recip"):
                        nc.vector.reciprocal(rc[:, i, qt:qt + 1],
                                             dn[:, qt, 64:65])
                    nc.vector.tensor_scalar_mul(
                        vst[:, qt, 2 * hp + i, :],
                        po[:, i, qt * 65:qt * 65 + 64],
                        rc[:, i, qt:qt + 1])

            def norm_all():
                for i in range(2):
                    dn = po[:, i, 0:260].rearrange("p (qt c) -> p qt c", c=65)
                    with nc.allow_low_precision(reason="softmax recip"):
                        nc.vector.reciprocal(rc[:, i, :], dn[:, 0:4, 64:65])
                    for qt in range(4):
                        nc.vector.tensor_scalar_mul(
                            vst[:, qt, 2 * hp + i, :],
                            po[:, i, qt * 65:qt * 65 + 64],
                            rc[:, i, qt:qt + 1])

            po = ps.tile([128, 2, 512], F32, tag="po", name=f"po{qc}{hp}", bufs=1)
            rc = spool.tile([128, 2, 4], F32, tag="rc", name=f"rc{qc}{hp}")
            es = dict(pre) if pre else {}
            if 0 not in es:
                es[0] = sc(0)
            if n_ki > 1 and 1 not in es:
                es[1] = sc(1)
            pre_next = None
            for ki in range(n_ki):
                if inline:
                    for fn in inline.get(ki, ()):
                        fn()
                if need and ki in need:
                    queue.drain_to(need[ki])
                av(ki, es.pop(ki))
                if ki + 2 < n_ki:
                    es[ki + 2] = sc(ki + 2)
                if next_hook and ki == n_ki - _HOOKLAG:
                    pre_next = next_hook()
                if tail and ki >= n_ki - 4:
                    qt = ki - (n_ki - 4)
                    norm_qt(qt)
                    tr_piece(qc, qt, dhs=(0,), copy_eng=nc.scalar)
                    tr_piece(qc, qt, dhs=(1,), copy_eng=nc.vector)
                else:
                    queue.drain_frac(per_step)
            if tail:
                for mp in range(4):
                    op_tail(qc, mp)
            else:
                norm_all()
            return pre_next

        # ---- chunk 0: hp0 qk + v0/v1 pass-major so PE has work while
        # pass-2/3 bytes stream in and first scores start ASAP ----
        for p in range(3):
            qk_pass(0, 0, p, tag="s")
            qk_pass(0, 2, p, tag="s")
            v_pass(0, 0, p)
            v_pass(0, 1, p)
        qk_bias(0, 0)
        qk_bias(0, 2)
        v_bias(0, 0)
        v_bias(0, 1)

        vst_tiles[0] = spool.tile([128, 4, 4, 64], BF16, tag="vst", name="vs0")
        # chunk 0's v tiles run inline between AVs; only qk hp1 is queued
        for mt in (1, 3):
            queue.push(lambda mt=mt: qk_tile(0, mt), f"qk0{mt}")
        push_qkv_early(1)
        inline0 = {ki: [lambda ki=ki: v_tile(0, ki)] for ki in (2, 3)}

        def hook_for(qc2, hp2, drains):
            """Pre-emit drains + the first two scores of group (qc2, hp2)."""
            def h():
                for d in drains:
                    queue.drain_to(d)
                es = {0: sc_of(qc2, hp2, 0)}
                if 4 * qc2 + 4 > 1:
                    es[1] = sc_of(qc2, hp2, 1)
                return es
            return h

        pre = attn_group(0, 0, inline=inline0,
                         per_step=_PACE[0] * queue.remaining() / 4,
                         next_hook=hook_for(0, 1, ["qk03"]))

        for qc in range(SC):
            n_ki = 4 * qc + 4
            if qc > 0:
                vst_tiles[qc] = spool.tile([128, 4, 4, 64], BF16, tag="vst",
                                           name=f"vs{qc}")
                push_qkv_late(qc)
                if qc + 1 < SC:
                    push_qkv_early(qc + 1)
                push_post(qc - 1)
                # scores need this chunk's qT/kT hp0; AV(ki) needs vn[ki]
                queue.drain_to(f"qk{qc}2")
                need = {max(0, 4 * qc + j - 2): f"v{qc}{j}" for j in range(4)}
                f0 = _PACE[3] if qc == SC - 1 else _PACE[1]
                pre = attn_group(qc, 0, need=need, pre=pre,
                                 per_step=f0 * queue.remaining() / n_ki,
                                 next_hook=hook_for(qc, 1, [f"qk{qc}3"]))
            # hp1 needs this chunk's mt=1,3 projections emitted first
            queue.drain_to(f"qk{qc}3")
            if qc < SC - 1:
                nh = hook_for(qc + 1, 0, [f"qk{qc + 1}2"])
                pre = attn_group(qc, 1, pre=pre, next_hook=nh,
                                 per_step=_PACE[2] * queue.remaining() / n_ki)
            else:
                attn_group(qc, 1, tail=True, pre=pre,
                           per_step=queue.remaining() / (n_ki - 4))

        queue.flush()
        if dbg:
            nc.sync.dma_start(d_vst, vst_tiles[0][:])
            nc.sync.dma_start(d_qT, qT[:])
            nc.sync.dma_start(d_kT, kT[:])
            nc.sync.dma_start(d_vn, vn[:])
            nc.sync.dma_start(d_vnT, vnT[:])

    if fix_waits:
        _fix_sync_waits(nc)
    return nc


def _get_nc():
    if "nc" not in _CACHE:
        _CACHE["nc"] = _build()
    return _CACHE["nc"]


def _dr_layout(xb):
    """[S, 1024] -> [128, 4, 2, S]: p=partition, kp=k-tile-pair, sl=slot."""
    return np.ascontiguousarray(
        xb.T.reshape(4, 2, 128, xb.shape[0]).transpose(2, 0, 1, 3))


def kernel(x, W_qkv, b_qkv, W_out, b_out):
    x = np.asarray(x, np.float32)
    W_qkv = np.asarray(W_qkv, np.float32)
    b_qkv = np.asarray(b_qkv, np.float32)
    W_out = np.asarray(W_out, np.float32)
    b_out = np.asarray(b_out, np.float32)

    nc = _get_nc()

    kk = np.arange(128)[:, None]
    qq = np.arange(128)[None, :]
    dmask = (-1e9 * np.eye(128, dtype=np.float32)).astype(BF)
    utri = (kk > qq).astype(BF)      # [r, q] = 1 where r > q
    identb = np.eye(128, dtype=np.float32).astype(BF)

    in_maps = []
    for c in range(N_CORES):
        b, g = divmod(c, 4)
        heads = [4 * g + i for i in range(HL)]

        xb = x[b]                                        # [S, 1024]
        xr = _dr_layout(xb)
        x8 = xr.astype(E4)
        x8l = ((xr - x8.astype(np.float32)) * 8.0).astype(E4)
        xs8_a = (xr * 0.125).astype(E4)

        # qk weight m-tiles: mt0=q-hp0, mt1=q-hp1, mt2=k-hp0, mt3=k-hp1
        # out-col within tile = 64*i + dd  (i head-in-pair, dd hd index)
        wq = np.zeros((1024, 4, 128), np.float32)
        bqv = np.zeros((128, 4), np.float32)
        for mt in range(4):
            t, hp = divmod(mt, 2)       # t: 0=q, 1=k
            for i in range(2):
                h = heads[2 * hp + i]
                cols = h * 192 + 64 * t + np.arange(64)
                wq[:, mt, 64 * i:64 * i + 64] = W_qkv[:, cols]
                bqv[64 * i:64 * i + 64, mt] = b_qkv[cols]
        # mt axis stored as [0,2,1,3]; [1024, mt, 128] -> [128(p), mt, kp, sl, 128]
        wq = wq[:, [0, 2, 1, 3], :]
        wq = wq.reshape(4, 2, 128, 4, 128).transpose(2, 3, 0, 1, 4)
        wq8 = wq.astype(E4)
        wq8l = ((wq - wq8.astype(np.float32)) * 8.0).astype(E4)
        wq8s = (wq * 0.125).astype(E4)

        # v weights: col = 64*h + dd
        wv = np.zeros((1024, 256), np.float32)
        bvv = np.zeros((4, 64), np.float32)
        for hh in range(4):
            cols = heads[hh] * 192 + 128 + np.arange(64)
            wv[:, 64 * hh:64 * hh + 64] = W_qkv[:, cols]
            bvv[hh] = b_qkv[cols]
        wv = wv.reshape(4, 2, 128, 256).transpose(2, 0, 1, 3)
        wv8 = wv.astype(E4)
        wv8l = ((wv - wv8.astype(np.float32)) * 8.0).astype(E4)
        wv8s = (wv * 0.125).astype(E4)
        bv2 = np.broadcast_to(bvv[None], (128, 4, 64))

        wo = W_out[g * 256:(g + 1) * 256, :]             # [256, 1024]
        wob = wo.reshape(2, 128, D).transpose(1, 0, 2).astype(BF)

        in_maps.append({
            "xq8": x8,
            "xq8l": x8l,
            "xs8": xs8_a,
            "wq8": np.ascontiguousarray(wq8),
            "wq8l": np.ascontiguousarray(wq8l),
            "wq8s": np.ascontiguousarray(wq8s),
            "wv8": np.ascontiguousarray(wv8),
            "wv8l": np.ascontiguousarray(wv8l),
            "wv8s": np.ascontiguousarray(wv8s),
            "woutb": np.ascontiguousarray(wob),
            "bq": np.ascontiguousarray(bqv),
            "bv": np.ascontiguousarray(bv2),
            "dmsk": np.ascontiguousarray(dmask),
            "utri": np.ascontiguousarray(utri),
            "identb": identb,
        })

    _CACHE["in_maps"] = in_maps
    res = bass_utils.run_bass_kernel_spmd(nc, in_maps, core_ids=list(range(N_CORES)))

    out = np.zeros((B, S, D), np.float32)
    for c in range(N_CORES):
        b = c // 4
        oT = np.asarray(res.results[c]["outT"]).astype(np.float32)
        out[b] += oT.transpose(1, 0, 2).reshape(D, S).T
    out += b_out
    return out


# revision 6
# speedup vs baseline: 1.0740x; 1.0000x over previous
"""Multi-head causal attention (B=2, S=2048, D=1024, H=16) on 8 TRN2 NeuronCores.

Sharding: core c handles batch b = c // 4 and local head group g = c % 4
(global heads 4g..4g+3).  Each core computes its heads' QKV projections,
causal attention, and a partial output projection; host sums the 4 partials
per batch and adds b_out.

v3 design, 120.3us TimelineSim (v2 baseline 129.2us):
  - Score-ahead pipelining: scores for ki+2 are emitted before AV(ki) in PE
    program order, so ACT (exp) runs back-to-back instead of ping-ponging
    with PE.  exp is the per-ki long pole (1024 els x 0.83ns vs PE 644ns).
  - Group-boundary pre-emit: the next (qc, hp) group's first two
    score+exp tiles are emitted inside the current group's tail (next_hook)
    so ACT has no bubble across hp/chunk transitions.
  - Causal mask folded into the scores psum accumulation as a PE matmul:
    diag(-1e9) @ strict-upper-tri accumulated before the f32r score matmul.
    exp(-1.25e8) = 0, so the post-exp DVE mask multiply is gone.
  - Psum pending-zero folded into the first AV matmul of each bank
    (start=True zeroes the bank) -- the zro dummy matmuls are gone.
  - Startup: weight mt axis stored [0,2,1,3] so hp0 slices are single DMAs,
    DMAs ordered by first use (few and large: HWDGE is a single global
    ~630ns/DMA device); chunk-0 hp0 QK + v0/v1 projections run pass-major
    so PE has work while pass-2/3 bytes stream in.
  - Output DMAs fused per head-pair (one [128,2,512] DMA per two m tiles).
  - Last chunk tail: per-qt normalize/transpose as each AV accumulation
    stops (ki = 12+qt); out-proj pairs use the freed s-tag psum slots for
    ring depth 4 with copies round-robin DVE/ACT.
  - Explicit drain-ordering (labels) replaces pacing-only correctness.
  - NOTE: GPSIMD cannot access PSUM on TRN2 (BIR verifier) -- all
    psum->sbuf moves must be on DVE or ACT.
"""

from contextlib import ExitStack

import numpy as np
import ml_dtypes

import concourse.bass as bass
import concourse.mybir as mybir
import concourse.tile as tile
from concourse import bass_utils

F32 = mybir.dt.float32
F32R = mybir.dt.float32r
BF16 = mybir.dt.bfloat16
FP8 = mybir.dt.float8e4
EXP = mybir.ActivationFunctionType.Exp
COPY = mybir.ActivationFunctionType.Copy
DR = mybir.MatmulPerfMode.DoubleRow

E4 = ml_dtypes.float8_e4m3
BF = ml_dtypes.bfloat16

B, S, D, H = 2, 2048, 1024, 16
HD = D // H          # 64
HL = 4               # heads per core
N_CORES = 8
SC = S // 512        # 4 q-chunks of 512
KT = S // 128        # 16 k-tiles of 128
MTX = {0: 0, 2: 1, 1: 2, 3: 3}  # mt -> stored position (hp0 pair first)

_CACHE = {}
_PACE = [0.4, 0.5, 0.6, 0.4]
_HOOKLAG = 2


def _round_f32r(x: np.ndarray) -> np.ndarray:
    """Round f32 to fp32r (11-bit mantissa, RNE) on host."""
    u = np.ascontiguousarray(x, dtype=np.float32).view(np.uint32)
    frac = u & np.uint32(0x00000FFF)
    base = u & np.uint32(0xFFFFF000)
    bit = np.uint32(0x00000800)
    lsb = np.uint32(0x00001000)
    roundup = (frac > bit) | ((frac == bit) & ((u & lsb) != 0))
    return np.where(roundup, base + lsb, base).view(np.float32)


_NO_HOIST = {
    "AllEngineBarrier",
    "EventSemaphore",
    "UnconditionalBranch",
    "CompareAndBranch",
    "BranchHint",
    "IndirectBranch",
    "Halt",
    "Call",
    "OverlayCall",
    "NoOp",
}


def _fix_sync_waits(nc):
    """walrus codegen holds only one sync-wait per engine instruction; hoist
    excess waits onto same-engine NoOps inserted right before."""
    for fn in nc.m.functions:
        for blk in fn.blocks:
            insts = blk.instructions
            out = []
            changed = False
            for inst in insts:
                si = inst.sync_info
                if si is not None and inst.opcode not in _NO_HOIST:
                    waits = list(si.on_wait)
                    if len(waits) > 1:
                        for j, w in enumerate(waits[:-1]):
                            nop = mybir.InstNoOp(name=f"{inst.name}-wfix{j}")
                            nop.engine = inst.engine
                            nop.sync_info = mybir.SyncInfo(on_wait=[w], on_update=[])
                            out.append(nop)
                        inst.sync_info = mybir.SyncInfo(
                            on_wait=[waits[-1]], on_update=list(si.on_update)
                        )
                        changed = True
                out.append(inst)
            if changed:
                blk.instructions = out


class _Q:
    """Emission-time work queue with credit pacing + forced ordering."""

    def __init__(self):
        self.items = []      # (fn, label)
        self.qi = 0
        self.credit = 0.0

    def push(self, fn, label=None):
        self.items.append((fn, label))

    def remaining(self):
        return len(self.items) - self.qi

    def _emit_one(self):
        fn, _ = self.items[self.qi]
        self.qi += 1
        fn()

    def drain_frac(self, frac):
        self.credit += frac
        while self.qi < len(self.items) and self.qi < self.credit:
            self._emit_one()

    def drain_to(self, label):
        """Emit everything up to and including the piece tagged `label`."""
        done = any(lb == label for _, lb in self.items[: self.qi])
        if done:
            return
        while self.qi < len(self.items):
            lb = self.items[self.qi][1]
            self._emit_one()
            self.credit = max(self.credit, self.qi)
            if lb == label:
                return
        raise KeyError(f"label {label} not found in queue")

    def flush(self):
        while self.qi < len(self.items):
            self._emit_one()
        self.credit = self.qi


def _build(fix_waits=True, dbg=False):
    nc = bass.Bass("TRN2", target_bir_lowering=False, debug=False,
                   num_devices=N_CORES)
    if dbg:
        d_qT = nc.dram_tensor("d_qT", [128, 2, S], F32R, kind="ExternalOutput").ap()
        d_kT = nc.dram_tensor("d_kT", [128, 2, S], F32R, kind="ExternalOutput").ap()
        d_vn = nc.dram_tensor("d_vn", [128, KT, 4, 65], BF16,
                              kind="ExternalOutput").ap()
        d_vst = nc.dram_tensor("d_vst", [128, 4, 4, 64], BF16,
                               kind="ExternalOutput").ap()
        d_vnT = nc.dram_tensor("d_vnT", [128, 2, S], BF16,
                               kind="ExternalOutput").ap()

    # x in fp8 hi / lo*8 / /8 copies, [128, kp, sl, S]
    xq8 = nc.dram_tensor("xq8", [128, 4, 2, S], FP8, kind="ExternalInput").ap()
    xq8l = nc.dram_tensor("xq8l", [128, 4, 2, S], FP8, kind="ExternalInput").ap()
    xs8 = nc.dram_tensor("xs8", [128, 4, 2, S], FP8, kind="ExternalInput").ap()
    # qk weights mt-major: [128, mt, kp, sl, 128]
    wq8 = nc.dram_tensor("wq8", [128, 4, 4, 2, 128], FP8, kind="ExternalInput").ap()
    wq8l = nc.dram_tensor("wq8l", [128, 4, 4, 2, 128], FP8, kind="ExternalInput").ap()
    wq8s = nc.dram_tensor("wq8s", [128, 4, 4, 2, 128], FP8, kind="ExternalInput").ap()
    wv8 = nc.dram_tensor("wv8", [128, 4, 2, 256], FP8, kind="ExternalInput").ap()
    wv8l = nc.dram_tensor("wv8l", [128, 4, 2, 256], FP8, kind="ExternalInput").ap()
    wv8s = nc.dram_tensor("wv8s", [128, 4, 2, 256], FP8, kind="ExternalInput").ap()
    woutb = nc.dram_tensor("woutb", [128, 2, D], BF16, kind="ExternalInput").ap()
    bq = nc.dram_tensor("bq", [128, 4], F32, kind="ExternalInput").ap()
    bv = nc.dram_tensor("bv", [128, 4, 64], F32, kind="ExternalInput").ap()
    dmsk = nc.dram_tensor("dmsk", [128, 128], BF16, kind="ExternalInput").ap()
    utri = nc.dram_tensor("utri", [128, 128], BF16, kind="ExternalInput").ap()
    identb = nc.dram_tensor("identb", [128, 128], BF16, kind="ExternalInput").ap()
    outT = nc.dram_tensor("outT", [128, 8, S], BF16, kind="ExternalOutput").ap()

    with tile.TileContext(nc) as tc, ExitStack() as ctx:
        persist = ctx.enter_context(tc.tile_pool(name="persist", bufs=1))
        xpool = ctx.enter_context(tc.tile_pool(name="xp", bufs=3))
        epool = ctx.enter_context(tc.tile_pool(name="ep", bufs=8))
        spool = ctx.enter_context(tc.tile_pool(name="stp", bufs=3))
        opool = ctx.enter_context(tc.tile_pool(name="op", bufs=6))
        # psum (8 banks): s 2x2-bank (sp / startup pq), po 1x2-bank,
        # q1 2x1-bank (pv/pq/pu/tr churn)
        ps = ctx.enter_context(tc.tile_pool(name="ps", bufs=2, space="PSUM"))

        wq_sb = persist.tile([128, 4, 4, 2, 128], FP8, tag="wq")
        wql_sb = persist.tile([128, 4, 4, 2, 128], FP8, tag="wql")
        wqs_sb = persist.tile([128, 4, 4, 2, 128], FP8, tag="wqs")
        wv_sb = persist.tile([128, 4, 2, 256], FP8, tag="wv")
        wvl_sb = persist.tile([128, 4, 2, 256], FP8, tag="wvl")
        wvs_sb = persist.tile([128, 4, 2, 256], FP8, tag="wvs")
        wo_sb = persist.tile([128, 2, D], BF16, tag="wo")
        bq_sb = persist.tile([128, 4], F32, tag="bq")
        bv_sb = persist.tile([128, 4, 64], F32, tag="bv")
        dm_sb = persist.tile([128, 128], BF16, tag="dm")
        ut_sb = persist.tile([128, 128], BF16, tag="ut")
        id_sb = persist.tile([128, 128], BF16, tag="id")
        qT = persist.tile([128, 2, S], F32R, tag="qT")
        kT = persist.tile([128, 2, S], F32R, tag="kT")
        vn = persist.tile([128, KT, 4, 65], BF16, tag="vn")
        vnT = persist.tile([128, 2, S], BF16, tag="vnT")

        # ---- startup DMAs, ordered by first use ----
        # weight mt axis is stored in order [0, 2, 1, 3] so the hp0 pair
        # (mt 0 and 2) is one contiguous 256KB DMA.
        xc0 = xpool.tile([128, 4, 2, 512], FP8, tag="xc", name="xc0")
        xl0 = xpool.tile([128, 4, 2, 512], FP8, tag="xl", name="xl0")
        xs0 = xpool.tile([128, 4, 2, 512], FP8, tag="xs", name="xs0")
        # pass 1: wq mt0/mt2 + xc0, split fine for first-byte latency
        nc.scalar.dma_start(wq_sb[:, 0:1], wq8[:, 0:1])
        nc.sync.dma_start(xc0[:, 0:2], xq8[:, 0:2, :, 0:512])
        nc.scalar.dma_start(wq_sb[:, 1:2], wq8[:, 1:2])
        nc.sync.dma_start(xc0[:, 2:4], xq8[:, 2:4, :, 0:512])
        nc.scalar.dma_start(wv_sb[:], wv8)
        # pass 2: wql mt0/mt2 + xs0
        nc.scalar.dma_start(wql_sb[:, 0:2], wq8l[:, 0:2])
        nc.sync.dma_start(xs0[:], xs8[:, :, :, 0:512])
        nc.scalar.dma_start(wvl_sb[:], wv8l)
        # pass 3: wqs mt0/mt2 + xl0
        nc.scalar.dma_start(wqs_sb[:, 0:2], wq8s[:, 0:2])
        nc.sync.dma_start(xl0[:], xq8l[:, :, :, 0:512])
        nc.sync.dma_start(bq_sb[:], bq)
        nc.sync.dma_start(dm_sb[:], dmsk)
        nc.sync.dma_start(ut_sb[:], utri)
        nc.scalar.dma_start(wvs_sb[:], wv8s)
        nc.sync.dma_start(bv_sb[:], bv)
        # hp1 qk weights (mt 1 and 3 = stored positions 2:4)
        nc.scalar.dma_start(wq_sb[:, 2:4], wq8[:, 2:4])
        nc.scalar.dma_start(wql_sb[:, 2:4], wq8l[:, 2:4])
        nc.scalar.dma_start(wqs_sb[:, 2:4], wq8s[:, 2:4])
        nc.scalar.dma_start(id_sb[:], identb)
        nc.scalar.dma_start(wo_sb[:], woutb)
        # ones column of vn (softmax denominators) via memset, not DMA
        nc.vector.memset(vn[:, :, :, 64:65], 1.0)
        xtiles = {0: (xc0, xl0, xs0)}

        def qkv_dma(qc):
            qs = slice(qc * 512, (qc + 1) * 512)
            xc = xpool.tile([128, 4, 2, 512], FP8, tag="xc", name=f"xc{qc}")
            xl = xpool.tile([128, 4, 2, 512], FP8, tag="xl", name=f"xl{qc}")
            xs = xpool.tile([128, 4, 2, 512], FP8, tag="xs", name=f"xs{qc}")
            nc.sync.dma_start(xc[:], xq8[:, :, :, qs])
            nc.sync.dma_start(xl[:], xq8l[:, :, :, qs])
            nc.sync.dma_start(xs[:], xs8[:, :, :, qs])
            xtiles[qc] = (xc, xl, xs)

        pq_tiles = {}

        def qk_pass(qc, mt, p, tag="q1"):
            """One error-compensation pass (4 DR matmuls) of a q/k tile."""
            xc, xl, xs = xtiles[qc]
            if p == 0:
                pq_tiles[(qc, mt)] = ps.tile([128, 512], F32, tag=tag,
                                             name=f"pq{qc}{mt}")
            pq = pq_tiles[(qc, mt)]
            wsb, xsb = [(wq_sb, xc), (wql_sb, xs), (wqs_sb, xl)][p]
            mtx = MTX[mt]
            for kp in range(4):
                nc.tensor.matmul(
                    pq[:], wsb[:, mtx, kp, :, :], xsb[:, kp, :, :],
                    start=(p == 0 and kp == 0), stop=(p == 2 and kp == 3),
                    perf_mode=DR)

        def qk_bias(qc, mt):
            qs = slice(qc * 512, (qc + 1) * 512)
            pq = pq_tiles.pop((qc, mt))
            dst = (qT if mt < 2 else kT)[:, mt % 2, qs]
            nc.vector.tensor_scalar_add(dst, pq[:], bq_sb[:, mt:mt + 1])

        def qk_tile(qc, mt):
            for p in range(3):
                qk_pass(qc, mt, p)
            qk_bias(qc, mt)

        pv_tiles = {}

        def v_pass(qc, j, p, tag="q1"):
            xc, xl, xs = xtiles[qc]
            if p == 0:
                pv_tiles[(qc, j)] = ps.tile([128, 256], F32, tag=tag,
                                            name=f"pv{qc}{j}")
            pv = pv_tiles[(qc, j)]
            wsb, xsb = [(wv_sb, xc), (wvl_sb, xs), (wvs_sb, xl)][p]
            for kp in range(4):
                nc.tensor.matmul(
                    pv[:], xsb[:, kp, :, j * 128:(j + 1) * 128],
                    wsb[:, kp, :, :],
                    start=(p == 0 and kp == 0), stop=(p == 2 and kp == 3),
                    perf_mode=DR)

        def v_bias(qc, j):
            st = 4 * qc + j
            pv = pv_tiles.pop((qc, j))
            nc.vector.tensor_add(
                vn[:, st, :, 0:64],
                pv[:].rearrange("p (h d) -> p h d", h=4),
                bv_sb[:])

        def v_tile(qc, j):
            st = 4 * qc + j
            xc, xl, xs = xtiles[qc]
            pv = ps.tile([128, 256], F32, tag="q1", name=f"pv{qc}{j}")
            passes = [(wv_sb, xc), (wvl_sb, xs), (wvs_sb, xl)]
            i = 0
            for wsb, xsb in passes:
                for kp in range(4):
                    nc.tensor.matmul(
                        pv[:], xsb[:, kp, :, j * 128:(j + 1) * 128],
                        wsb[:, kp, :, :],
                        start=(i == 0), stop=(i == 11), perf_mode=DR)
                    i += 1
            nc.vector.tensor_add(
                vn[:, st, :, 0:64],
                pv[:].rearrange("p (h d) -> p h d", h=4),
                bv_sb[:])

        vst_tiles = {}

        def tr_piece(qc, qt, dhs=(0, 1), copy_eng=None, via_dma=False):
            vst = vst_tiles[qc]
            for dh in dhs:
                dst = vnT[:, dh, qc * 512 + qt * 128:qc * 512 + (qt + 1) * 128]
                if via_dma:
                    # SBUF->SBUF crossbar transpose on the DMA path: no PE
                    # or DVE time, fine for latency-insensitive pieces
                    nc.sync.dma_start_transpose(dst, vst[:, qt, 2 * dh:2 * dh + 2, :])
                    continue
                ptr = ps.tile([128, 128], BF16, tag="q1", name=f"tr{qc}{qt}{dh}")
                nc.tensor.matmul(ptr[:], vst[:, qt, 2 * dh:2 * dh + 2, :],
                                 id_sb[:], is_transpose=True)
                eng = copy_eng or nc.vector
                if eng is nc.scalar:
                    eng.copy(dst, ptr[:])
                else:
                    eng.tensor_copy(dst, ptr[:])

        def op_pair(qc, mp, tags=("q1", "q1"), engs=None, split_dma=False):
            """Out-proj for heads-pair mp (m = 2mp, 2mp+1): 4 matmuls, two
            psum->sbuf copies, ONE fused output DMA (HWDGE is a single
            global device at ~630ns per DMA, so fewer DMAs win)."""
            qs = slice(qc * 512, (qc + 1) * 512)
            ou = opool.tile([128, 2, 512], BF16, tag="ou", name=f"ou{qc}{mp}")
            for j, m in enumerate((2 * mp, 2 * mp + 1)):
                pu = ps.tile([128, 512], F32, tag=tags[j], name=f"pu{qc}{m}")
                for t in range(2):
                    nc.tensor.matmul(pu[:], wo_sb[:, t, m * 128:(m + 1) * 128],
                                     vnT[:, t, qs], start=(t == 0), stop=(t == 1))
                eng = engs[j] if engs else nc.vector
                if eng is nc.scalar:
                    eng.copy(ou[:, j], pu[:])
                else:
                    eng.tensor_copy(ou[:, j], pu[:])
                if split_dma:
                    (nc.sync if j == 0 else nc.scalar).dma_start(
                        outT[:, m, qs], ou[:, j])
            if not split_dma:
                (nc.sync if mp % 2 == 0 else nc.scalar).dma_start(
                    outT[:, 2 * mp:2 * mp + 2, qs], ou[:])

        def op_tail(qc, mp):
            """Tail out-proj pair: pu psum uses the q1 and (now free) s tags;
            copies round-robin DVE/ACT/GPSIMD to pipeline behind PE."""
            engs = [(nc.vector, nc.scalar), (nc.vector, nc.scalar),
                    (nc.scalar, nc.vector), (nc.scalar, nc.vector)][mp]
            op_pair(qc, mp, tags=("q1", "s"), engs=engs, split_dma=(mp == 3))

        queue = _Q()

        def push_qkv_late(c):
            for j in (2, 3):
                queue.push(lambda c=c, j=j: v_tile(c, j), f"v{c}{j}")
            for mt in (1, 3):
                queue.push(lambda c=c, mt=mt: qk_tile(c, mt), f"qk{c}{mt}")

        def push_qkv_early(c):
            queue.push(lambda c=c: qkv_dma(c), f"dma{c}")
            for mt in (0, 2):
                queue.push(lambda c=c, mt=mt: qk_tile(c, mt), f"qk{c}{mt}")
            for j in (0, 1):
                queue.push(lambda c=c, j=j: v_tile(c, j), f"v{c}{j}")

        def push_post(c):
            for qt in range(4):
                queue.push(lambda c=c, qt=qt: tr_piece(c, qt), f"tr{c}{qt}")
            for mp in range(4):
                queue.push(lambda c=c, mp=mp: op_pair(c, mp), f"op{c}{mp}")

        def sc_of(qc, hp, ki):
            """Scores + exp for one k-tile of group (qc, hp): causal-mask
            matmul (diag tiles), f32r score matmuls, ACT exp -> e tile."""
            j = ki - 4 * qc
            o_exp = max(0, 128 * j)
            o_sc = min(o_exp, 256)  # f32r moving dim must be >= 256
            sp = ps.tile([128, 2, 512], F32, tag="s", name=f"sp{qc}{hp}{ki}")
            for i in range(2):
                vp = 64 * i
                if j >= 0:
                    # causal mask: psum[k, q] -= 1e9 * [k > q] on the
                    # diagonal block, via diag(-1e9) @ strict-upper-tri
                    nc.tensor.matmul(
                        sp[:, i, o_exp:o_exp + 128], dm_sb[:], ut_sb[:],
                        start=True, stop=False, skip_group_check=True)
                nc.tensor.matmul(
                    sp[:, i, o_sc:512],
                    kT[vp:vp + 64, hp, ki * 128:(ki + 1) * 128],
                    qT[vp:vp + 64, hp, qc * 512 + o_sc:(qc + 1) * 512],
                    start=(j < 0), stop=True, tile_position=(vp, 0),
                    skip_group_check=True)
            e = epool.tile([128, 2, 512], BF16, tag="e", name=f"e{qc}{hp}{ki}")
            nc.scalar.activation(e[:, :, o_exp:512], sp[:, :, o_exp:512],
                                 EXP, scale=0.125)
            return e

        def attn_group(qc, hp, inline=None, per_step=0.0, tail=False,
                       need=None, pre=None, next_hook=None):
            """Attention for group (qc, hp) with score-ahead pipelining.

            inline: optional dict ki -> [fn] of pieces emitted right before
            AV(ki) (used for chunk 0's v tiles).  need: dict ki -> queue
            label that must be emitted before AV(ki) (vn dependencies).
            pre: e tiles {0,1} pre-emitted by the previous group's tail.
            next_hook: called at ki == n_ki-2 to pre-emit the NEXT group's
            first scores so ACT has no bubble at the group boundary;
            its return value is returned.  tail=True pipelines the last
            chunk's normalize/transpose/out-proj per qt.
            """
            vst = vst_tiles[qc]
            n_ki = 4 * qc + 4

            def sc(ki):
                return sc_of(qc, hp, ki)

            def av(ki, e):
                j = ki - 4 * qc
                for i in range(2):
                    for qt in range(max(0, j), 4):
                        nc.tensor.matmul(
                            po[:, i, qt * 65:qt * 65 + 65],
                            e[:, i, qt * 128:(qt + 1) * 128],
                            vn[:, ki, 2 * hp + i, :],
                            start=(ki == 0 and qt == 0),
                            stop=(ki == 4 * qc + qt),
                            skip_group_check=True)

            def norm_qt(qt):
                for i in range(2):
                    dn = po[:, i, 0:260].rearrange("p (qt c) -> p qt c", c=65)
                    with nc.allow_low_precision(reason="softmax recip"):
                        nc.vector.reciprocal(rc[:, i, qt:qt + 1],
                                             dn[:, qt, 64:65])
                    nc.vector.tensor_scalar_mul(
                        vst[:, qt, 2 * hp + i, :],
                        po[:, i, qt * 65:qt * 65 + 64],
                        rc[:, i, qt:qt + 1])

            def norm_all():
                for i in range(2):
                    dn = po[:, i, 0:260].rearrange("p (qt c) -> p qt c", c=65)
                    with nc.allow_low_precision(reason="softmax recip"):
                        nc.vector.reciprocal(rc[:, i, :], dn[:, 0:4, 64:65])
                    for qt in range(4):
                        nc.vector.tensor_scalar_mul(
                            vst[:, qt, 2 * hp + i, :],
                            po[:, i, qt * 65:qt * 65 + 64],
                            rc[:, i, qt:qt + 1])

            po = ps.tile([128, 2, 512], F32, tag="po", name=f"po{qc}{hp}", bufs=1)
            rc = spool.tile([128, 2, 4], F32, tag="rc", name=f"rc{qc}{hp}")
            es = dict(pre) if pre else {}
            if 0 not in es:
                es[0] = sc(0)
            if n_ki > 1 and 1 not in es:
                es[1] = sc(1)
            pre_next = None
            for ki in range(n_ki):
                if inline:
                    for fn in inline.get(ki, ()):
                        fn()
                if need and ki in need:
                    queue.drain_to(need[ki])
                av(ki, es.pop(ki))
                if ki + 2 < n_ki:
                    es[ki + 2] = sc(ki + 2)
                if next_hook and ki == n_ki - _HOOKLAG:
                    pre_next = next_hook()
                if tail and ki >= n_ki - 4:
                    qt = ki - (n_ki - 4)
                    norm_qt(qt)
                    tr_piece(qc, qt, dhs=(0,), copy_eng=nc.scalar)
                    tr_piece(qc, qt, dhs=(1,), copy_eng=nc.vector)
                else:
                    queue.drain_frac(per_step)
            if tail:
                for mp in range(4):
                    op_tail(qc, mp)
            else:
                norm_all()
            return pre_next

        # ---- chunk 0: hp0 qk + v0/v1 pass-major so PE has work while
        # pass-2/3 bytes stream in and first scores start ASAP ----
        for p in range(3):
            qk_pass(0, 0, p, tag="s")
            qk_pass(0, 2, p, tag="s")
            v_pass(0, 0, p)
            v_pass(0, 1, p)
        qk_bias(0, 0)
        qk_bias(0, 2)
        v_bias(0, 0)
        v_bias(0, 1)

        vst_tiles[0] = spool.tile([128, 4, 4, 64], BF16, tag="vst", name="vs0")
        # chunk 0's v tiles run inline between AVs; only qk hp1 is queued
        for mt in (1, 3):
            queue.push(lambda mt=mt: qk_tile(0, mt), f"qk0{mt}")
        push_qkv_early(1)
        inline0 = {ki: [lambda ki=ki: v_tile(0, ki)] for ki in (2, 3)}

        def hook_for(qc2, hp2, drains):
            """Pre-emit drains + the first two scores of group (qc2, hp2)."""
            def h():
                for d in drains:
                    queue.drain_to(d)
                es = {0: sc_of(qc2, hp2, 0)}
                if 4 * qc2 + 4 > 1:
                    es[1] = sc_of(qc2, hp2, 1)
                return es
            return h

        pre = attn_group(0, 0, inline=inline0,
                         per_step=_PACE[0] * queue.remaining() / 4,
                         next_hook=hook_for(0, 1, ["qk03"]))

        for qc in range(SC):
            n_ki = 4 * qc + 4
            if qc > 0:
                vst_tiles[qc] = spool.tile([128, 4, 4, 64], BF16, tag="vst",
                                           name=f"vs{qc}")
                push_qkv_late(qc)
                if qc + 1 < SC:
                    push_qkv_early(qc + 1)
                push_post(qc - 1)
                # scores need this chunk's qT/kT hp0; AV(ki) needs vn[ki]
                queue.drain_to(f"qk{qc}2")
                need = {max(0, 4 * qc + j - 2): f"v{qc}{j}" for j in range(4)}
                f0 = _PACE[3] if qc == SC - 1 else _PACE[1]
                pre = attn_group(qc, 0, need=need, pre=pre,
                                 per_step=f0 * queue.remaining() / n_ki,
                                 next_hook=hook_for(qc, 1, [f"qk{qc}3"]))
            # hp1 needs this chunk's mt=1,3 projections emitted first
            queue.drain_to(f"qk{qc}3")
            if qc < SC - 1:
                nh = hook_for(qc + 1, 0, [f"qk{qc + 1}2"])
                pre = attn_group(qc, 1, pre=pre, next_hook=nh,
                                 per_step=_PACE[2] * queue.remaining() / n_ki)
            else:
                attn_group(qc, 1, tail=True, pre=pre,
                           per_step=queue.remaining() / (n_ki - 4))

        queue.flush()
        if dbg:
            nc.sync.dma_start(d_vst, vst_tiles[0][:])
            nc.sync.dma_start(d_qT, qT[:])
            nc.sync.dma_start(d_kT, kT[:])
            nc.sync.dma_start(d_vn, vn[:])
            nc.sync.dma_start(d_vnT, vnT[:])

    if fix_waits:
        _fix_sync_waits(nc)
    return nc


def _get_nc():
    if "nc" not in _CACHE:
        _CACHE["nc"] = _build()
    return _CACHE["nc"]


def _dr_layout(xb):
    """[S, 1024] -> [128, 4, 2, S]: p=partition, kp=k-tile-pair, sl=slot."""
    return np.ascontiguousarray(
        xb.T.reshape(4, 2, 128, xb.shape[0]).transpose(2, 0, 1, 3))


def kernel(x, W_qkv, b_qkv, W_out, b_out):
    x = np.asarray(x, np.float32)
    W_qkv = np.asarray(W_qkv, np.float32)
    b_qkv = np.asarray(b_qkv, np.float32)
    W_out = np.asarray(W_out, np.float32)
    b_out = np.asarray(b_out, np.float32)

    nc = _get_nc()

    kk = np.arange(128)[:, None]
    qq = np.arange(128)[None, :]
    dmask = (-1e9 * np.eye(128, dtype=np.float32)).astype(BF)
    utri = (kk > qq).astype(BF)      # [r, q] = 1 where r > q
    identb = np.eye(128, dtype=np.float32).astype(BF)

    in_maps = []
    for c in range(N_CORES):
        b, g = divmod(c, 4)
        heads = [4 * g + i for i in range(HL)]

        xb = x[b]                                        # [S, 1024]
        xr = _dr_layout(xb)
        x8 = xr.astype(E4)
        x8l = ((xr - x8.astype(np.float32)) * 8.0).astype(E4)
        xs8_a = (xr * 0.125).astype(E4)

        # qk weight m-tiles: mt0=q-hp0, mt1=q-hp1, mt2=k-hp0, mt3=k-hp1
        # out-col within tile = 64*i + dd  (i head-in-pair, dd hd index)
        wq = np.zeros((1024, 4, 128), np.float32)
        bqv = np.zeros((128, 4), np.float32)
        for mt in range(4):
            t, hp = divmod(mt, 2)       # t: 0=q, 1=k
            for i in range(2):
                h = heads[2 * hp + i]
                cols = h * 192 + 64 * t + np.arange(64)
                wq[:, mt, 64 * i:64 * i + 64] = W_qkv[:, cols]
                bqv[64 * i:64 * i + 64, mt] = b_qkv[cols]
        # mt axis stored as [0,2,1,3]; [1024, mt, 128] -> [128(p), mt, kp, sl, 128]
        wq = wq[:, [0, 2, 1, 3], :]
        wq = wq.reshape(4, 2, 128, 4, 128).transpose(2, 3, 0, 1, 4)
        wq8 = wq.astype(E4)
        wq8l = ((wq - wq8.astype(np.float32)) * 8.0).astype(E4)
        wq8s = (wq * 0.125).astype(E4)

        # v weights: col = 64*h + dd
        wv = np.zeros((1024, 256), np.float32)
        bvv = np.zeros((4, 64), np.float32)
        for hh in range(4):
            cols = heads[hh] * 192 + 128 + np.arange(64)
            wv[:, 64 * hh:64 * hh + 64] = W_qkv[:, cols]
            bvv[hh] = b_qkv[cols]
        wv = wv.reshape(4, 2, 128, 256).transpose(2, 0, 1, 3)
        wv8 = wv.astype(E4)
        wv8l = ((wv - wv8.astype(np.float32)) * 8.0).astype(E4)
        wv8s = (wv * 0.125).astype(E4)
        bv2 = np.broadcast_to(bvv[None], (128, 4, 64))

        wo = W_out[g * 256:(g + 1) * 256, :]             # [256, 1024]
        wob = wo.reshape(2, 128, D).transpose(1, 0, 2).astype(BF)

        in_maps.append({
            "xq8": x8,
            "xq8l": x8l,
            "xs8": xs8_a,
            "wq8": np.ascontiguousarray(wq8),
            "wq8l": np.ascontiguousarray(wq8l),
            "wq8s": np.ascontiguousarray(wq8s),
            "wv8": np.ascontiguousarray(wv8),
            "wv8l": np.ascontiguousarray(wv8l),
            "wv8s": np.ascontiguousarray(wv8s),
            "woutb": np.ascontiguousarray(wob),
            "bq": np.ascontiguousarray(bqv),
            "bv": np.ascontiguousarray(bv2),
            "dmsk": np.ascontiguousarray(dmask),
            "utri": np.ascontiguousarray(utri),
            "identb": identb,
        })

    _CACHE["in_maps"] = in_maps
    res = bass_utils.run_bass_kernel_spmd(nc, in_maps, core_ids=list(range(N_CORES)))

    out = np.zeros((B, S, D), np.float32)
    for c in range(N_CORES):
        b = c // 4
        oT = np.asarray(res.results[c]["outT"]).astype(np.float32)
        out[b] += oT.transpose(1, 0, 2).reshape(D, S).T
    out += b_out
    return out


# revision 7
# speedup vs baseline: 1.0778x; 1.0036x over previous
"""Multi-head causal attention (B=2, S=2048, D=1024, H=16) on 8 TRN2 NeuronCores.

Sharding: core c handles batch b = c // 4 and local head group g = c % 4
(global heads 4g..4g+3).  Each core computes its heads' QKV projections,
causal attention, and a partial output projection; host sums the 4 partials
per batch and adds b_out.

v3 design, 119.9us TimelineSim (v2 baseline 129.2us):
  - Score-ahead pipelining: scores for ki+2 are emitted before AV(ki) in PE
    program order, so ACT (exp) runs back-to-back instead of ping-ponging
    with PE.  exp is the per-ki long pole (1024 els x 0.83ns vs PE 644ns).
  - Group-boundary pre-emit: the next (qc, hp) group's first two
    score+exp tiles are emitted inside the current group's tail (next_hook)
    so ACT has no bubble across hp/chunk transitions.
  - Causal mask folded into the scores psum accumulation as a PE matmul:
    diag(-1e9) @ strict-upper-tri accumulated before the f32r score matmul.
    exp(-1.25e8) = 0, so the post-exp DVE mask multiply is gone.
  - Psum pending-zero folded into the first AV matmul of each bank
    (start=True zeroes the bank) -- the zro dummy matmuls are gone.
  - Startup: weight mt axis stored [0,2,1,3] so hp0 slices are single DMAs,
    DMAs ordered by first use (few and large: HWDGE is a single global
    ~630ns/DMA device); chunk-0 hp0 QK + v0/v1 projections run pass-major
    so PE has work while pass-2/3 bytes stream in.
  - Output DMAs fused per head-pair (one [128,2,512] DMA per two m tiles).
  - Last chunk tail: per-qt normalize/transpose as each AV accumulation
    stops (ki = 12+qt); out-proj pairs use the freed s-tag psum slots for
    ring depth 4 with copies round-robin DVE/ACT.
  - Explicit drain-ordering (labels) replaces pacing-only correctness.
  - NOTE: GPSIMD cannot access PSUM on TRN2 (BIR verifier) -- all
    psum->sbuf moves must be on DVE or ACT.
"""

from contextlib import ExitStack

import numpy as np
import ml_dtypes

import concourse.bass as bass
import concourse.mybir as mybir
import concourse.tile as tile
from concourse import bass_utils

F32 = mybir.dt.float32
F32R = mybir.dt.float32r
BF16 = mybir.dt.bfloat16
FP8 = mybir.dt.float8e4
EXP = mybir.ActivationFunctionType.Exp
COPY = mybir.ActivationFunctionType.Copy
DR = mybir.MatmulPerfMode.DoubleRow

E4 = ml_dtypes.float8_e4m3
BF = ml_dtypes.bfloat16

B, S, D, H = 2, 2048, 1024, 16
HD = D // H          # 64
HL = 4               # heads per core
N_CORES = 8
SC = S // 512        # 4 q-chunks of 512
KT = S // 128        # 16 k-tiles of 128
MTX = {0: 0, 2: 1, 1: 2, 3: 3}  # mt -> stored position (hp0 pair first)

_CACHE = {}
_PACE = [0.4, 0.5, 0.6, 0.4]
_HOOKLAG = 2
_TDIV = 11


def _round_f32r(x: np.ndarray) -> np.ndarray:
    """Round f32 to fp32r (11-bit mantissa, RNE) on host."""
    u = np.ascontiguousarray(x, dtype=np.float32).view(np.uint32)
    frac = u & np.uint32(0x00000FFF)
    base = u & np.uint32(0xFFFFF000)
    bit = np.uint32(0x00000800)
    lsb = np.uint32(0x00001000)
    roundup = (frac > bit) | ((frac == bit) & ((u & lsb) != 0))
    return np.where(roundup, base + lsb, base).view(np.float32)


_NO_HOIST = {
    "AllEngineBarrier",
    "EventSemaphore",
    "UnconditionalBranch",
    "CompareAndBranch",
    "BranchHint",
    "IndirectBranch",
    "Halt",
    "Call",
    "OverlayCall",
    "NoOp",
}


def _fix_sync_waits(nc):
    """walrus codegen holds only one sync-wait per engine instruction; hoist
    excess waits onto same-engine NoOps inserted right before."""
    for fn in nc.m.functions:
        for blk in fn.blocks:
            insts = blk.instructions
            out = []
            changed = False
            for inst in insts:
                si = inst.sync_info
                if si is not None and inst.opcode not in _NO_HOIST:
                    waits = list(si.on_wait)
                    if len(waits) > 1:
                        for j, w in enumerate(waits[:-1]):
                            nop = mybir.InstNoOp(name=f"{inst.name}-wfix{j}")
                            nop.engine = inst.engine
                            nop.sync_info = mybir.SyncInfo(on_wait=[w], on_update=[])
                            out.append(nop)
                        inst.sync_info = mybir.SyncInfo(
                            on_wait=[waits[-1]], on_update=list(si.on_update)
                        )
                        changed = True
                out.append(inst)
            if changed:
                blk.instructions = out


class _Q:
    """Emission-time work queue with credit pacing + forced ordering."""

    def __init__(self):
        self.items = []      # (fn, label)
        self.qi = 0
        self.credit = 0.0

    def push(self, fn, label=None):
        self.items.append((fn, label))

    def remaining(self):
        return len(self.items) - self.qi

    def _emit_one(self):
        fn, _ = self.items[self.qi]
        self.qi += 1
        fn()

    def drain_frac(self, frac):
        self.credit += frac
        while self.qi < len(self.items) and self.qi < self.credit:
            self._emit_one()

    def drain_to(self, label):
        """Emit everything up to and including the piece tagged `label`."""
        done = any(lb == label for _, lb in self.items[: self.qi])
        if done:
            return
        while self.qi < len(self.items):
            lb = self.items[self.qi][1]
            self._emit_one()
            self.credit = max(self.credit, self.qi)
            if lb == label:
                return
        raise KeyError(f"label {label} not found in queue")

    def flush(self):
        while self.qi < len(self.items):
            self._emit_one()
        self.credit = self.qi


def _build(fix_waits=True, dbg=False):
    nc = bass.Bass("TRN2", target_bir_lowering=False, debug=False,
                   num_devices=N_CORES)
    if dbg:
        d_qT = nc.dram_tensor("d_qT", [128, 2, S], F32R, kind="ExternalOutput").ap()
        d_kT = nc.dram_tensor("d_kT", [128, 2, S], F32R, kind="ExternalOutput").ap()
        d_vn = nc.dram_tensor("d_vn", [128, KT, 4, 65], BF16,
                              kind="ExternalOutput").ap()
        d_vst = nc.dram_tensor("d_vst", [128, 4, 4, 64], BF16,
                               kind="ExternalOutput").ap()
        d_vnT = nc.dram_tensor("d_vnT", [128, 2, S], BF16,
                               kind="ExternalOutput").ap()

    # x in fp8 hi / lo*8 / /8 copies, [128, kp, sl, S]
    xq8 = nc.dram_tensor("xq8", [128, 4, 2, S], FP8, kind="ExternalInput").ap()
    xq8l = nc.dram_tensor("xq8l", [128, 4, 2, S], FP8, kind="ExternalInput").ap()
    xs8 = nc.dram_tensor("xs8", [128, 4, 2, S], FP8, kind="ExternalInput").ap()
    # qk weights mt-major: [128, mt, kp, sl, 128]
    wq8 = nc.dram_tensor("wq8", [128, 4, 4, 2, 128], FP8, kind="ExternalInput").ap()
    wq8l = nc.dram_tensor("wq8l", [128, 4, 4, 2, 128], FP8, kind="ExternalInput").ap()
    wq8s = nc.dram_tensor("wq8s", [128, 4, 4, 2, 128], FP8, kind="ExternalInput").ap()
    wv8 = nc.dram_tensor("wv8", [128, 4, 2, 256], FP8, kind="ExternalInput").ap()
    wv8l = nc.dram_tensor("wv8l", [128, 4, 2, 256], FP8, kind="ExternalInput").ap()
    wv8s = nc.dram_tensor("wv8s", [128, 4, 2, 256], FP8, kind="ExternalInput").ap()
    woutb = nc.dram_tensor("woutb", [128, 2, D], BF16, kind="ExternalInput").ap()
    bq = nc.dram_tensor("bq", [128, 4], F32, kind="ExternalInput").ap()
    bv = nc.dram_tensor("bv", [128, 4, 64], F32, kind="ExternalInput").ap()
    dmsk = nc.dram_tensor("dmsk", [128, 128], BF16, kind="ExternalInput").ap()
    utri = nc.dram_tensor("utri", [128, 128], BF16, kind="ExternalInput").ap()
    identb = nc.dram_tensor("identb", [128, 128], BF16, kind="ExternalInput").ap()
    outT = nc.dram_tensor("outT", [128, 8, S], BF16, kind="ExternalOutput").ap()

    with tile.TileContext(nc) as tc, ExitStack() as ctx:
        persist = ctx.enter_context(tc.tile_pool(name="persist", bufs=1))
        xpool = ctx.enter_context(tc.tile_pool(name="xp", bufs=3))
        epool = ctx.enter_context(tc.tile_pool(name="ep", bufs=8))
        spool = ctx.enter_context(tc.tile_pool(name="stp", bufs=3))
        opool = ctx.enter_context(tc.tile_pool(name="op", bufs=6))
        # psum (8 banks): s 2x2-bank (sp / startup pq), po 1x2-bank,
        # q1 2x1-bank (pv/pq/pu/tr churn)
        ps = ctx.enter_context(tc.tile_pool(name="ps", bufs=2, space="PSUM"))

        wq_sb = persist.tile([128, 4, 4, 2, 128], FP8, tag="wq")
        wql_sb = persist.tile([128, 4, 4, 2, 128], FP8, tag="wql")
        wqs_sb = persist.tile([128, 4, 4, 2, 128], FP8, tag="wqs")
        wv_sb = persist.tile([128, 4, 2, 256], FP8, tag="wv")
        wvl_sb = persist.tile([128, 4, 2, 256], FP8, tag="wvl")
        wvs_sb = persist.tile([128, 4, 2, 256], FP8, tag="wvs")
        wo_sb = persist.tile([128, 2, D], BF16, tag="wo")
        bq_sb = persist.tile([128, 4], F32, tag="bq")
        bv_sb = persist.tile([128, 4, 64], F32, tag="bv")
        dm_sb = persist.tile([128, 128], BF16, tag="dm")
        ut_sb = persist.tile([128, 128], BF16, tag="ut")
        id_sb = persist.tile([128, 128], BF16, tag="id")
        qT = persist.tile([128, 2, S], F32R, tag="qT")
        kT = persist.tile([128, 2, S], F32R, tag="kT")
        vn = persist.tile([128, KT, 4, 65], BF16, tag="vn")
        vnT = persist.tile([128, 2, S], BF16, tag="vnT")

        # ---- startup DMAs, ordered by first use ----
        # weight mt axis is stored in order [0, 2, 1, 3] so the hp0 pair
        # (mt 0 and 2) is one contiguous 256KB DMA.
        xc0 = xpool.tile([128, 4, 2, 512], FP8, tag="xc", name="xc0")
        xl0 = xpool.tile([128, 4, 2, 512], FP8, tag="xl", name="xl0")
        xs0 = xpool.tile([128, 4, 2, 512], FP8, tag="xs", name="xs0")
        # pass 1: wq mt0/mt2 + xc0, split fine for first-byte latency
        nc.scalar.dma_start(wq_sb[:, 0:1], wq8[:, 0:1])
        nc.sync.dma_start(xc0[:, 0:2], xq8[:, 0:2, :, 0:512])
        nc.scalar.dma_start(wq_sb[:, 1:2], wq8[:, 1:2])
        nc.sync.dma_start(xc0[:, 2:4], xq8[:, 2:4, :, 0:512])
        nc.scalar.dma_start(wv_sb[:], wv8)
        # pass 2: wql mt0/mt2 + xs0
        nc.scalar.dma_start(wql_sb[:, 0:2], wq8l[:, 0:2])
        nc.sync.dma_start(xs0[:], xs8[:, :, :, 0:512])
        nc.scalar.dma_start(wvl_sb[:], wv8l)
        # pass 3: wqs mt0/mt2 + xl0
        nc.scalar.dma_start(wqs_sb[:, 0:2], wq8s[:, 0:2])
        nc.sync.dma_start(xl0[:], xq8l[:, :, :, 0:512])
        nc.sync.dma_start(bq_sb[:], bq)
        nc.sync.dma_start(dm_sb[:], dmsk)
        nc.sync.dma_start(ut_sb[:], utri)
        nc.scalar.dma_start(wvs_sb[:], wv8s)
        nc.sync.dma_start(bv_sb[:], bv)
        # hp1 qk weights (mt 1 and 3 = stored positions 2:4)
        nc.scalar.dma_start(wq_sb[:, 2:4], wq8[:, 2:4])
        nc.scalar.dma_start(wql_sb[:, 2:4], wq8l[:, 2:4])
        nc.scalar.dma_start(wqs_sb[:, 2:4], wq8s[:, 2:4])
        nc.scalar.dma_start(id_sb[:], identb)
        nc.scalar.dma_start(wo_sb[:], woutb)
        # ones column of vn (softmax denominators) via memset, not DMA
        nc.vector.memset(vn[:, :, :, 64:65], 1.0)
        xtiles = {0: (xc0, xl0, xs0)}

        def qkv_dma(qc):
            qs = slice(qc * 512, (qc + 1) * 512)
            xc = xpool.tile([128, 4, 2, 512], FP8, tag="xc", name=f"xc{qc}")
            xl = xpool.tile([128, 4, 2, 512], FP8, tag="xl", name=f"xl{qc}")
            xs = xpool.tile([128, 4, 2, 512], FP8, tag="xs", name=f"xs{qc}")
            nc.sync.dma_start(xc[:], xq8[:, :, :, qs])
            nc.sync.dma_start(xl[:], xq8l[:, :, :, qs])
            nc.sync.dma_start(xs[:], xs8[:, :, :, qs])
            xtiles[qc] = (xc, xl, xs)

        pq_tiles = {}

        def qk_pass(qc, mt, p, tag="q1"):
            """One error-compensation pass (4 DR matmuls) of a q/k tile."""
            xc, xl, xs = xtiles[qc]
            if p == 0:
                pq_tiles[(qc, mt)] = ps.tile([128, 512], F32, tag=tag,
                                             name=f"pq{qc}{mt}")
            pq = pq_tiles[(qc, mt)]
            wsb, xsb = [(wq_sb, xc), (wql_sb, xs), (wqs_sb, xl)][p]
            mtx = MTX[mt]
            for kp in range(4):
                nc.tensor.matmul(
                    pq[:], wsb[:, mtx, kp, :, :], xsb[:, kp, :, :],
                    start=(p == 0 and kp == 0), stop=(p == 2 and kp == 3),
                    perf_mode=DR)

        def qk_bias(qc, mt):
            qs = slice(qc * 512, (qc + 1) * 512)
            pq = pq_tiles.pop((qc, mt))
            dst = (qT if mt < 2 else kT)[:, mt % 2, qs]
            nc.vector.tensor_scalar_add(dst, pq[:], bq_sb[:, mt:mt + 1])

        def qk_tile(qc, mt):
            for p in range(3):
                qk_pass(qc, mt, p)
            qk_bias(qc, mt)

        pv_tiles = {}

        def v_pass(qc, j, p, tag="q1"):
            xc, xl, xs = xtiles[qc]
            if p == 0:
                pv_tiles[(qc, j)] = ps.tile([128, 256], F32, tag=tag,
                                            name=f"pv{qc}{j}")
            pv = pv_tiles[(qc, j)]
            wsb, xsb = [(wv_sb, xc), (wvl_sb, xs), (wvs_sb, xl)][p]
            for kp in range(4):
                nc.tensor.matmul(
                    pv[:], xsb[:, kp, :, j * 128:(j + 1) * 128],
                    wsb[:, kp, :, :],
                    start=(p == 0 and kp == 0), stop=(p == 2 and kp == 3),
                    perf_mode=DR)

        def v_bias(qc, j):
            st = 4 * qc + j
            pv = pv_tiles.pop((qc, j))
            nc.vector.tensor_add(
                vn[:, st, :, 0:64],
                pv[:].rearrange("p (h d) -> p h d", h=4),
                bv_sb[:])

        def v_tile(qc, j):
            st = 4 * qc + j
            xc, xl, xs = xtiles[qc]
            pv = ps.tile([128, 256], F32, tag="q1", name=f"pv{qc}{j}")
            passes = [(wv_sb, xc), (wvl_sb, xs), (wvs_sb, xl)]
            i = 0
            for wsb, xsb in passes:
                for kp in range(4):
                    nc.tensor.matmul(
                        pv[:], xsb[:, kp, :, j * 128:(j + 1) * 128],
                        wsb[:, kp, :, :],
                        start=(i == 0), stop=(i == 11), perf_mode=DR)
                    i += 1
            nc.vector.tensor_add(
                vn[:, st, :, 0:64],
                pv[:].rearrange("p (h d) -> p h d", h=4),
                bv_sb[:])

        vst_tiles = {}

        def tr_piece(qc, qt, dhs=(0, 1), copy_eng=None, via_dma=False):
            vst = vst_tiles[qc]
            for dh in dhs:
                dst = vnT[:, dh, qc * 512 + qt * 128:qc * 512 + (qt + 1) * 128]
                if via_dma:
                    # SBUF->SBUF crossbar transpose on the DMA path: no PE
                    # or DVE time, fine for latency-insensitive pieces
                    nc.sync.dma_start_transpose(dst, vst[:, qt, 2 * dh:2 * dh + 2, :])
                    continue
                ptr = ps.tile([128, 128], BF16, tag="q1", name=f"tr{qc}{qt}{dh}")
                nc.tensor.matmul(ptr[:], vst[:, qt, 2 * dh:2 * dh + 2, :],
                                 id_sb[:], is_transpose=True)
                eng = copy_eng or nc.vector
                if eng is nc.scalar:
                    eng.copy(dst, ptr[:])
                else:
                    eng.tensor_copy(dst, ptr[:])

        def op_pair(qc, mp, tags=("q1", "q1"), engs=None, split_dma=False):
            """Out-proj for heads-pair mp (m = 2mp, 2mp+1): 4 matmuls, two
            psum->sbuf copies, ONE fused output DMA (HWDGE is a single
            global device at ~630ns per DMA, so fewer DMAs win)."""
            qs = slice(qc * 512, (qc + 1) * 512)
            ou = opool.tile([128, 2, 512], BF16, tag="ou", name=f"ou{qc}{mp}")
            for j, m in enumerate((2 * mp, 2 * mp + 1)):
                pu = ps.tile([128, 512], F32, tag=tags[j], name=f"pu{qc}{m}")
                for t in range(2):
                    nc.tensor.matmul(pu[:], wo_sb[:, t, m * 128:(m + 1) * 128],
                                     vnT[:, t, qs], start=(t == 0), stop=(t == 1))
                eng = engs[j] if engs else nc.vector
                if eng is nc.scalar:
                    eng.copy(ou[:, j], pu[:])
                else:
                    eng.tensor_copy(ou[:, j], pu[:])
                if split_dma:
                    (nc.sync if j == 0 else nc.scalar).dma_start(
                        outT[:, m, qs], ou[:, j])
            if not split_dma:
                (nc.sync if mp % 2 == 0 else nc.scalar).dma_start(
                    outT[:, 2 * mp:2 * mp + 2, qs], ou[:])

        def op_tail(qc, mp):
            """Tail out-proj pair: pu psum uses the q1 and (now free) s tags;
            copies round-robin DVE/ACT/GPSIMD to pipeline behind PE."""
            engs = [(nc.vector, nc.scalar), (nc.vector, nc.scalar),
                    (nc.scalar, nc.vector), (nc.scalar, nc.vector)][mp]
            op_pair(qc, mp, tags=("q1", "s"), engs=engs, split_dma=(mp == 3))

        queue = _Q()

        def push_qkv_late(c):
            for j in (2, 3):
                queue.push(lambda c=c, j=j: v_tile(c, j), f"v{c}{j}")
            for mt in (1, 3):
                queue.push(lambda c=c, mt=mt: qk_tile(c, mt), f"qk{c}{mt}")

        def push_qkv_early(c):
            queue.push(lambda c=c: qkv_dma(c), f"dma{c}")
            for mt in (0, 2):
                queue.push(lambda c=c, mt=mt: qk_tile(c, mt), f"qk{c}{mt}")
            for j in (0, 1):
                queue.push(lambda c=c, j=j: v_tile(c, j), f"v{c}{j}")

        def push_post(c):
            for qt in range(4):
                queue.push(lambda c=c, qt=qt: tr_piece(c, qt), f"tr{c}{qt}")
            for mp in range(4):
                queue.push(lambda c=c, mp=mp: op_pair(c, mp), f"op{c}{mp}")

        def sc_of(qc, hp, ki):
            """Scores + exp for one k-tile of group (qc, hp): causal-mask
            matmul (diag tiles), f32r score matmuls, ACT exp -> e tile."""
            j = ki - 4 * qc
            o_exp = max(0, 128 * j)
            o_sc = min(o_exp, 256)  # f32r moving dim must be >= 256
            sp = ps.tile([128, 2, 512], F32, tag="s", name=f"sp{qc}{hp}{ki}")
            for i in range(2):
                vp = 64 * i
                if j >= 0:
                    # causal mask: psum[k, q] -= 1e9 * [k > q] on the
                    # diagonal block, via diag(-1e9) @ strict-upper-tri
                    nc.tensor.matmul(
                        sp[:, i, o_exp:o_exp + 128], dm_sb[:], ut_sb[:],
                        start=True, stop=False, skip_group_check=True)
                nc.tensor.matmul(
                    sp[:, i, o_sc:512],
                    kT[vp:vp + 64, hp, ki * 128:(ki + 1) * 128],
                    qT[vp:vp + 64, hp, qc * 512 + o_sc:(qc + 1) * 512],
                    start=(j < 0), stop=True, tile_position=(vp, 0),
                    skip_group_check=True)
            e = epool.tile([128, 2, 512], BF16, tag="e", name=f"e{qc}{hp}{ki}")
            nc.scalar.activation(e[:, :, o_exp:512], sp[:, :, o_exp:512],
                                 EXP, scale=0.125)
            return e

        def attn_group(qc, hp, inline=None, per_step=0.0, tail=False,
                       need=None, pre=None, next_hook=None):
            """Attention for group (qc, hp) with score-ahead pipelining.

            inline: optional dict ki -> [fn] of pieces emitted right before
            AV(ki) (used for chunk 0's v tiles).  need: dict ki -> queue
            label that must be emitted before AV(ki) (vn dependencies).
            pre: e tiles {0,1} pre-emitted by the previous group's tail.
            next_hook: called at ki == n_ki-2 to pre-emit the NEXT group's
            first scores so ACT has no bubble at the group boundary;
            its return value is returned.  tail=True pipelines the last
            chunk's normalize/transpose/out-proj per qt.
            """
            vst = vst_tiles[qc]
            n_ki = 4 * qc + 4

            def sc(ki):
                return sc_of(qc, hp, ki)

            def av(ki, e):
                j = ki - 4 * qc
                for i in range(2):
                    for qt in range(max(0, j), 4):
                        nc.tensor.matmul(
                            po[:, i, qt * 65:qt * 65 + 65],
                            e[:, i, qt * 128:(qt + 1) * 128],
                            vn[:, ki, 2 * hp + i, :],
                            start=(ki == 0 and qt == 0),
                            stop=(ki == 4 * qc + qt),
                            skip_group_check=True)

            def norm_qt(qt):
                for i in range(2):
                    dn = po[:, i, 0:260].rearrange("p (qt c) -> p qt c", c=65)
                    with nc.allow_low_precision(reason="softmax recip"):
                        nc.vector.reciprocal(rc[:, i, qt:qt + 1],
                                             dn[:, qt, 64:65])
                    nc.vector.tensor_scalar_mul(
                        vst[:, qt, 2 * hp + i, :],
                        po[:, i, qt * 65:qt * 65 + 64],
                        rc[:, i, qt:qt + 1])

            def norm_all():
                for i in range(2):
                    dn = po[:, i, 0:260].rearrange("p (qt c) -> p qt c", c=65)
                    with nc.allow_low_precision(reason="softmax recip"):
                        nc.vector.reciprocal(rc[:, i, :], dn[:, 0:4, 64:65])
                    for qt in range(4):
                        nc.vector.tensor_scalar_mul(
                            vst[:, qt, 2 * hp + i, :],
                            po[:, i, qt * 65:qt * 65 + 64],
                            rc[:, i, qt:qt + 1])

            po = ps.tile([128, 2, 512], F32, tag="po", name=f"po{qc}{hp}", bufs=1)
            rc = spool.tile([128, 2, 4], F32, tag="rc", name=f"rc{qc}{hp}")
            es = dict(pre) if pre else {}
            if 0 not in es:
                es[0] = sc(0)
            if n_ki > 1 and 1 not in es:
                es[1] = sc(1)
            pre_next = None
            for ki in range(n_ki):
                if inline:
                    for fn in inline.get(ki, ()):
                        fn()
                if need and ki in need:
                    queue.drain_to(need[ki])
                av(ki, es.pop(ki))
                if ki + 2 < n_ki:
                    es[ki + 2] = sc(ki + 2)
                if next_hook and ki == n_ki - _HOOKLAG:
                    pre_next = next_hook()
                if tail and ki >= n_ki - 4:
                    qt = ki - (n_ki - 4)
                    norm_qt(qt)
                    tr_piece(qc, qt, dhs=(0,), copy_eng=nc.scalar)
                    tr_piece(qc, qt, dhs=(1,), copy_eng=nc.vector)
                else:
                    queue.drain_frac(per_step)
            if tail:
                for mp in range(4):
                    op_tail(qc, mp)
            else:
                norm_all()
            return pre_next

        # ---- chunk 0: hp0 qk + v0/v1 pass-major so PE has work while
        # pass-2/3 bytes stream in and first scores start ASAP ----
        for p in range(3):
            qk_pass(0, 0, p, tag="s")
            qk_pass(0, 2, p, tag="s")
            v_pass(0, 0, p)
            v_pass(0, 1, p)
        qk_bias(0, 0)
        qk_bias(0, 2)
        v_bias(0, 0)
        v_bias(0, 1)

        vst_tiles[0] = spool.tile([128, 4, 4, 64], BF16, tag="vst", name="vs0")
        # chunk 0's v tiles run inline between AVs; only qk hp1 is queued
        for mt in (1, 3):
            queue.push(lambda mt=mt: qk_tile(0, mt), f"qk0{mt}")
        push_qkv_early(1)
        inline0 = {ki: [lambda ki=ki: v_tile(0, ki)] for ki in (2, 3)}

        def hook_for(qc2, hp2, drains):
            """Pre-emit drains + the first two scores of group (qc2, hp2)."""
            def h():
                for d in drains:
                    queue.drain_to(d)
                es = {0: sc_of(qc2, hp2, 0)}
                if 4 * qc2 + 4 > 1:
                    es[1] = sc_of(qc2, hp2, 1)
                return es
            return h

        pre = attn_group(0, 0, inline=inline0,
                         per_step=_PACE[0] * queue.remaining() / 4,
                         next_hook=hook_for(0, 1, ["qk03"]))

        for qc in range(SC):
            n_ki = 4 * qc + 4
            if qc > 0:
                vst_tiles[qc] = spool.tile([128, 4, 4, 64], BF16, tag="vst",
                                           name=f"vs{qc}")
                push_qkv_late(qc)
                if qc + 1 < SC:
                    push_qkv_early(qc + 1)
                push_post(qc - 1)
                # scores need this chunk's qT/kT hp0; AV(ki) needs vn[ki]
                queue.drain_to(f"qk{qc}2")
                need = {max(0, 4 * qc + j - 2): f"v{qc}{j}" for j in range(4)}
                f0 = _PACE[3] if qc == SC - 1 else _PACE[1]
                pre = attn_group(qc, 0, need=need, pre=pre,
                                 per_step=f0 * queue.remaining() / n_ki,
                                 next_hook=hook_for(qc, 1, [f"qk{qc}3"]))
            # hp1 needs this chunk's mt=1,3 projections emitted first
            queue.drain_to(f"qk{qc}3")
            if qc < SC - 1:
                nh = hook_for(qc + 1, 0, [f"qk{qc + 1}2"])
                pre = attn_group(qc, 1, pre=pre, next_hook=nh,
                                 per_step=_PACE[2] * queue.remaining() / n_ki)
            else:
                attn_group(qc, 1, tail=True, pre=pre,
                           per_step=queue.remaining() / _TDIV)

        queue.flush()
        if dbg:
            nc.sync.dma_start(d_vst, vst_tiles[0][:])
            nc.sync.dma_start(d_qT, qT[:])
            nc.sync.dma_start(d_kT, kT[:])
            nc.sync.dma_start(d_vn, vn[:])
            nc.sync.dma_start(d_vnT, vnT[:])

    if fix_waits:
        _fix_sync_waits(nc)
    return nc


def _get_nc():
    if "nc" not in _CACHE:
        _CACHE["nc"] = _build()
    return _CACHE["nc"]


def _dr_layout(xb):
    """[S, 1024] -> [128, 4, 2, S]: p=partition, kp=k-tile-pair, sl=slot."""
    return np.ascontiguousarray(
        xb.T.reshape(4, 2, 128, xb.shape[0]).transpose(2, 0, 1, 3))


def kernel(x, W_qkv, b_qkv, W_out, b_out):
    x = np.asarray(x, np.float32)
    W_qkv = np.asarray(W_qkv, np.float32)
    b_qkv = np.asarray(b_qkv, np.float32)
    W_out = np.asarray(W_out, np.float32)
    b_out = np.asarray(b_out, np.float32)

    nc = _get_nc()

    kk = np.arange(128)[:, None]
    qq = np.arange(128)[None, :]
    dmask = (-1e9 * np.eye(128, dtype=np.float32)).astype(BF)
    utri = (kk > qq).astype(BF)      # [r, q] = 1 where r > q
    identb = np.eye(128, dtype=np.float32).astype(BF)

    in_maps = []
    for c in range(N_CORES):
        b, g = divmod(c, 4)
        heads = [4 * g + i for i in range(HL)]

        xb = x[b]                                        # [S, 1024]
        xr = _dr_layout(xb)
        x8 = xr.astype(E4)
        x8l = ((xr - x8.astype(np.float32)) * 8.0).astype(E4)
        xs8_a = (xr * 0.125).astype(E4)

        # qk weight m-tiles: mt0=q-hp0, mt1=q-hp1, mt2=k-hp0, mt3=k-hp1
        # out-col within tile = 64*i + dd  (i head-in-pair, dd hd index)
        wq = np.zeros((1024, 4, 128), np.float32)
        bqv = np.zeros((128, 4), np.float32)
        for mt in range(4):
            t, hp = divmod(mt, 2)       # t: 0=q, 1=k
            for i in range(2):
                h = heads[2 * hp + i]
                cols = h * 192 + 64 * t + np.arange(64)
                wq[:, mt, 64 * i:64 * i + 64] = W_qkv[:, cols]
                bqv[64 * i:64 * i + 64, mt] = b_qkv[cols]
        # mt axis stored as [0,2,1,3]; [1024, mt, 128] -> [128(p), mt, kp, sl, 128]
        wq = wq[:, [0, 2, 1, 3], :]
        wq = wq.reshape(4, 2, 128, 4, 128).transpose(2, 3, 0, 1, 4)
        wq8 = wq.astype(E4)
        wq8l = ((wq - wq8.astype(np.float32)) * 8.0).astype(E4)
        wq8s = (wq * 0.125).astype(E4)

        # v weights: col = 64*h + dd
        wv = np.zeros((1024, 256), np.float32)
        bvv = np.zeros((4, 64), np.float32)
        for hh in range(4):
            cols = heads[hh] * 192 + 128 + np.arange(64)
            wv[:, 64 * hh:64 * hh + 64] = W_qkv[:, cols]
            bvv[hh] = b_qkv[cols]
        wv = wv.reshape(4, 2, 128, 256).transpose(2, 0, 1, 3)
        wv8 = wv.astype(E4)
        wv8l = ((wv - wv8.astype(np.float32)) * 8.0).astype(E4)
        wv8s = (wv * 0.125).astype(E4)
        bv2 = np.broadcast_to(bvv[None], (128, 4, 64))

        wo = W_out[g * 256:(g + 1) * 256, :]             # [256, 1024]
        wob = wo.reshape(2, 128, D).transpose(1, 0, 2).astype(BF)

        in_maps.append({
            "xq8": x8,
            "xq8l": x8l,
            "xs8": xs8_a,
            "wq8": np.ascontiguousarray(wq8),
            "wq8l": np.ascontiguousarray(wq8l),
            "wq8s": np.ascontiguousarray(wq8s),
            "wv8": np.ascontiguousarray(wv8),
            "wv8l": np.ascontiguousarray(wv8l),
            "wv8s": np.ascontiguousarray(wv8s),
            "woutb": np.ascontiguousarray(wob),
            "bq": np.ascontiguousarray(bqv),
            "bv": np.ascontiguousarray(bv2),
            "dmsk": np.ascontiguousarray(dmask),
            "utri": np.ascontiguousarray(utri),
            "identb": identb,
        })

    _CACHE["in_maps"] = in_maps
    res = bass_utils.run_bass_kernel_spmd(nc, in_maps, core_ids=list(range(N_CORES)))

    out = np.zeros((B, S, D), np.float32)
    for c in range(N_CORES):
        b = c // 4
        oT = np.asarray(res.results[c]["outT"]).astype(np.float32)
        out[b] += oT.transpose(1, 0, 2).reshape(D, S).T
    out += b_out
    return out
